# revision 46
# baseline (speedup 1.0000x reference)
"""Chamfer-distance loss kernel for Trainium2 (8 NeuronCores, SPMD).

Problem: loss = chamfer(coarse, gt_pts) + alpha * chamfer(fine, gt_pts)
  coarse [8,1024,3], fine [8,8192,3], gt [8,3,8192] (channel-first), alpha scalar.
  chamfer(x,y) = mean_n min_m d(n,m) + mean_m min_n d(n,m), d = squared L2.

Sharding: data-parallel over batch - one batch element per NeuronCore.

Per-core pipeline (negated distances, DMA-transposed column path):
  - The PE produces NEGATED distances 128x512 at a time as a K=7 fp16 matmul:
      lhsT rows {x0,x1,x2, -x2hi,-x2lo, 1,1}
      rhs  rows {2y0,2y1,2y2, 1,1, -y2hi,-y2lo}
    so PSUM = -d in fp32 (|x|^2, |y|^2 at ~fp32 precision via fp16 hi/lo
    splits of presummed norms). All mins are maxes; the host negates at the
    end. Negation lets partition collapses use gpsimd partition_all_reduce
    (max) on the otherwise-idle Pool engine (supports add/max/absmax only).
  - Every (tile, m-half) produces S fp16 [128, 4096] via a cast pass split
    between ScalarE (ACTF.Copy) and VectorE (tensor_copy) - the only two
    engines with PSUM access; they are jointly the drain bottleneck.
  - Row direction: one tensor_scalar per (tile, half) at fp16 4x mode
    (op0=max vs -60000 = identity, op1=max into accum_out rowW[:, i, mh]).
  - Col direction:
      * fine m-lo half + head of m-hi: S is DMA-TRANSPOSED (XBAR, idle DMA
        engines) into TG[m_p, j, slot, n] for groups of GT=4 tiles (TG
        double-buffered); per (group, j) one tensor_scalar 4x accum-max over
        free=[4,128] yields uncontaminated per-m maxes at ~0.39 ns/elem
        instead of 0.52 (tensor_tensor 2x). Group flushes are emitted a few
        tiles late to give the DMA runway (engines execute in order).
      * fine m-hi tail + coarse: fp16 2x tensor_tensor running-max into acc
        tiles; partition collapse via Pool partition_all_reduce(max).
  - Scalar totals come from free-axis tensor_reduce(add) + Pool
    partition_all_reduce(add); no PE transposes or ones-matmuls.
  - HW-verified constraints: gpsimd/Pool tensor_tensor/tensor_scalar and DMA
    accum are ISA-rejected on TRN2; matmul psum out must be fp32; DVE 4x
    requires all-SBUF operands (arranged by the transposed-S path);
    partition_all_reduce and XBAR DMA transpose verified correct on HW.

Host does only O(N) prep and the final scalar arithmetic.
"""

import sys

sys.path.insert(0, "/opt/trn_rl_repo")

import numpy as np

B = 8
NF = 8192  # fine points
NC_ = 1024  # coarse points
M = 8192  # gt points
MH = M // 2  # m-half size
NJ = MH // 128  # 32 j-blocks per half

# --- tuning knobs ----------------------------------------------------------
GT = 4  # X-route group size (tiles per transposed group)
TG_BUFS = 2  # transposed-group buffer depth
CAST_DVE_MOD = 8  # every CAST_DVE_MOD-th cast group goes to VectorE
FLUSH_DELAY = 2  # tiles of DMA runway before a group flush
NJ2 = 18  # j-blocks of the m-hi half also X-routed (rest: tt route)

# --- module-level program cache -------------------------------------------
_PROGRAM = None
PROFILE = False
LAST_RESULTS = None


def _build_program():
    from concourse import bacc, bass, tile, bass_isa
    import concourse.mybir as mybir

    f16, f32 = mybir.dt.float16, mybir.dt.float32
    AL = mybir.AluOpType
    ACTF = mybir.ActivationFunctionType
    RED = bass_isa.ReduceOp

    nc = bacc.Bacc("TRN2", target_bir_lowering=False, debug=False, num_devices=B)

    xaug_f = nc.dram_tensor("xaug_f", [7, NF], f16, kind="ExternalInput")
    xaug_c = nc.dram_tensor("xaug_c", [7, NC_], f16, kind="ExternalInput")
    yaug_d = nc.dram_tensor("yaug", [7, M], f16, kind="ExternalInput")
    out_d = nc.dram_tensor("out", [1, 8], f32, kind="ExternalOutput")

    n_fine_tiles = NF // 128  # 64
    n_coarse_tiles = NC_ // 128  # 8
    assert n_fine_tiles % GT == 0

    cast_counter = [0]

    with tile.TileContext(nc) as tc:
        with (
            tc.tile_pool(name="const", bufs=1) as cpool,
            tc.tile_pool(name="s", bufs=3) as spool,
            tc.tile_pool(name="scr", bufs=2) as scrpool,
            tc.tile_pool(name="tg", bufs=TG_BUFS) as tgpool,
            tc.tile_pool(name="tg2", bufs=TG_BUFS) as tg2pool,
            tc.tile_pool(name="ct", bufs=2) as ctpool,
            tc.tile_pool(name="rr", bufs=1) as rrpool,
            tc.tile_pool(name="fin", bufs=1) as fpool,
            tc.tile_pool(name="ps", bufs=2, space=bass.MemorySpace.PSUM) as pspool,
        ):
            warm = cpool.tile([7, 512], f16)
            nc.gpsimd.memset(warm[:], 1.0)
            wsink = cpool.tile([1, 1], f32)
            for _w in range(6):
                wps = pspool.tile([128, 512], f32, name=f"wps{_w}", tag="ps")
                nc.tensor.matmul(wps[:], lhsT=warm[:, 0:128], rhs=warm[:, 0:512],
                                 start=True, stop=True)
                nc.vector.tensor_copy(wsink[:], wps[0:1, 0:1])
            Xf = cpool.tile([7, NF], f16)
            nc.sync.dma_start(Xf[:], xaug_f.ap())
            Xc = cpool.tile([7, NC_], f16)
            nc.sync.dma_start(Xc[:], xaug_c.ap())
            Y = cpool.tile([7, M], f16)
            nc.sync.dma_start(Y[:], yaug_d.ap())

            outb = cpool.tile([1, 8], f32)

            MT = MH - NJ2 * 128  # m-hi tail width kept on the tt route
            NJA = NJ + NJ2  # X-routed j-blocks per tile
            # per-family m-hi-tail col accumulators (tt route)
            accF = cpool.tile([128, MT], f16, name="accF") if MT else None
            accC = cpool.tile([128, MT], f16, name="accC") if MT else None
            rowWf = cpool.tile([128, n_fine_tiles, 2], f32)
            rowWc = cpool.tile([128, n_coarse_tiles, 2], f32)
            colWF = cpool.tile([128, NJA], f32)  # fine X-route per-m col maxes
            colWC = cpool.tile([128, NJA], f32)  # coarse X-route col maxes

            def make_S(Xa, i, mh):
                """matmuls + cast -> S fp16 [128, MH] (= -d), m-half mh."""
                S = spool.tile([128, MH], f16, tag="S")
                for g in range(2):
                    ps = pspool.tile([128, 2048], f32, tag="ps")
                    for j in range(4):
                        mlo = mh * MH + g * 2048 + j * 512
                        nc.tensor.matmul(
                            ps[:, j * 512 : (j + 1) * 512],
                            lhsT=Xa[:, i * 128 : (i + 1) * 128],
                            rhs=Y[:, mlo : mlo + 512],
                            start=True,
                            stop=True,
                        )
                    c = cast_counter[0]
                    cast_counter[0] += 1
                    if c % CAST_DVE_MOD == 3 or c < 1:
                        nc.vector.tensor_copy(S[:, g * 2048 : (g + 1) * 2048], ps[:])
                    else:
                        nc.scalar.activation(
                            S[:, g * 2048 : (g + 1) * 2048],
                            ps[:],
                            ACTF.Copy,
                            bias=0.0,
                            scale=1.0,
                        )
                return S

            def row_max(S, rowW, i, mh):
                scr = scrpool.tile([128, MH], f16, tag="scr")
                nc.vector.tensor_scalar(
                    out=scr[:],
                    in0=S[:],
                    scalar1=-60000.0,
                    scalar2=None,
                    op0=AL.max,
                    op1=AL.max,
                    accum_out=rowW[:, i, mh : mh + 1],
                )

            def scalar_out(red, oidx):
                """red [128, 1] f32 -> outb[0, oidx] via Pool partition sum."""
                pr = rrpool.tile([128, 1], f32, tag="par1")
                nc.gpsimd.partition_all_reduce(pr[:], red[:], channels=128,
                                               reduce_op=RED.add)
                nc.vector.tensor_copy(outb[0:1, oidx : oidx + 1], pr[0:1, 0:1])

            def collapse_sum(acc, oidx):
                """Pool-collapse acc [128, W] f16 over partitions (in 2048-wide
                chunks), then sum over m into outb[0, oidx]."""
                w = acc.shape[1]
                nchunks = (w + 2047) // 2048
                redc = fpool.tile([128, nchunks], f32, tag=f"redc{oidx}")
                for k in range(nchunks):
                    cw = min(2048, w - k * 2048)
                    rr = rrpool.tile([128, 2048], f32, tag="rr")
                    nc.gpsimd.partition_all_reduce(
                        rr[:, 0:cw], acc[:, k * 2048 : k * 2048 + cw],
                        channels=128, reduce_op=RED.max,
                    )
                    nc.vector.tensor_reduce(
                        out=redc[:, k : k + 1], in_=rr[:, 0:cw],
                        axis=mybir.AxisListType.X, op=AL.add,
                    )
                red = fpool.tile([128, 1], f32, tag=f"cred{oidx}")
                nc.vector.tensor_reduce(
                    out=red[:], in_=redc[:], axis=mybir.AxisListType.X, op=AL.add
                )
                nc.vector.tensor_copy(outb[0:1, oidx : oidx + 1], red[0:1, 0:1])

            # ---- unified tile stream: coarse first, then fine; every tile
            # X-routes m-lo fully and the head of m-hi; tails go through
            # tensor_tensor accumulators ----
            first_group = {"f": True, "c": True}

            def flush_group(TG, TG2, colW, fam, nS=GT):
                if first_group[fam]:
                    tgt = colW
                else:
                    tgt = ctpool.tile([128, NJA], f32, tag="colT")
                for j in range(NJ):
                    scr2 = scrpool.tile([128, GT * 128], f16, tag="scrj")
                    nc.vector.tensor_scalar(
                        out=scr2[:, 0 : nS * 128],
                        in0=TG[:, j, 0:nS, :],
                        scalar1=-60000.0,
                        scalar2=None,
                        op0=AL.max,
                        op1=AL.max,
                        accum_out=tgt[:, j : j + 1],
                    )
                for j in range(NJ2):
                    scr2 = scrpool.tile([128, GT * 128], f16, tag="scrj")
                    nc.vector.tensor_scalar(
                        out=scr2[:, 0 : nS * 128],
                        in0=TG2[:, j, 0:nS, :],
                        scalar1=-60000.0,
                        scalar2=None,
                        op0=AL.max,
                        op1=AL.max,
                        accum_out=tgt[:, NJ + j : NJ + j + 1],
                    )
                if tgt is not colW:
                    nc.vector.tensor_tensor(
                        out=colW[:], in0=colW[:], in1=tgt[:], op=AL.max
                    )
                first_group[fam] = False

            tiles = [("c", i) for i in range(n_coarse_tiles)] + [
                ("f", i) for i in range(n_fine_tiles)
            ]
            n_groups_full = len(tiles) // GT
            group_sizes = [GT] * n_groups_full
            assert sum(group_sizes) == len(tiles)
            gidx = [0]
            TG = TG2 = None
            in_group = 0
            step = 0
            pending = []  # (due_step, flush args) delayed for DMA runway
            for fam, i in tiles:
                Xa = Xf if fam == "f" else Xc
                rowW = rowWf if fam == "f" else rowWc
                acc = accF if fam == "f" else accC
                colW = colWF if fam == "f" else colWC
                # m-lo half: X route
                S = make_S(Xa, i, 0)
                row_max(S, rowW, i, 0)
                if in_group == 0:
                    TG = tgpool.tile([128, NJ, GT, 128], f16, tag="TG")
                    TG2 = tg2pool.tile(
                        [128, NJ2, GT, 128], f16, name="TG2", tag="TG2"
                    )
                nc.sync.dma_start(TG[:, :, in_group, :], S[:], transpose=True)
                # m-hi half: head X-routed, tail tt-routed
                S = make_S(Xa, i, 1)
                row_max(S, rowW, i, 1)
                nc.sync.dma_start(
                    TG2[:, :, in_group, :], S[:, 0 : NJ2 * 128], transpose=True
                )
                if MT:
                    if i == 0:
                        nc.vector.tensor_copy(acc[:], S[:, NJ2 * 128 : MH])
                    else:
                        nc.vector.tensor_tensor(
                            out=acc[:],
                            in0=acc[:],
                            in1=S[:, NJ2 * 128 : MH],
                            op=AL.max,
                        )
                in_group += 1
                gsz = group_sizes[gidx[0]]
                if in_group == gsz:
                    pending.append(
                        (step + 2 * FLUSH_DELAY, (TG, TG2, colW, fam, gsz))
                    )
                    in_group = 0
                    gidx[0] += 1
                step += 1
                while pending and pending[0][0] <= step:
                    flush_group(*pending.pop(0)[1])
                # mid-stream coarse tail collapse (Pool engine, async)
                if MT and fam == "f" and i == 2:
                    collapse_sum(accC, 7)

            # ---- finals ----
            def row_total(rowW, nT, oidx):
                rmax = fpool.tile([128, nT], f32, tag=f"rmax{oidx}")
                nc.vector.tensor_reduce(
                    out=rmax[:], in_=rowW[:], axis=mybir.AxisListType.X, op=AL.max
                )
                rsum = fpool.tile([128, 1], f32, tag=f"rsum{oidx}")
                nc.vector.tensor_reduce(
                    out=rsum[:], in_=rmax[:], axis=mybir.AxisListType.X, op=AL.add
                )
                scalar_out(rsum, oidx)

            # colW-independent finals first: the trailing group flushes wait
            # on DMA transposes, so this work overlaps them
            if MT:
                collapse_sum(accF, 4)
            else:
                nc.vector.memset(outb[0:1, 4:5], 0.0)
            row_total(rowWf, n_fine_tiles, 0)
            row_total(rowWc, n_coarse_tiles, 2)
            for _, args in pending:
                flush_group(*args)

            # X-route col sums
            cxr = fpool.tile([128, 1], f32, tag="cxr")
            nc.vector.tensor_reduce(
                out=cxr[:], in_=colWF[:], axis=mybir.AxisListType.X, op=AL.add
            )
            scalar_out(cxr, 3)
            cxc = fpool.tile([128, 1], f32, tag="cxc")
            nc.vector.tensor_reduce(
                out=cxc[:], in_=colWC[:], axis=mybir.AxisListType.X, op=AL.add
            )
            scalar_out(cxc, 6)
            if not MT:
                nc.vector.memset(outb[0:1, 7:8], 0.0)

            nc.vector.memset(outb[0:1, 1:2], 0.0)
            nc.vector.memset(outb[0:1, 5:6], 0.0)
            nc.sync.dma_start(out_d.ap(), outb[:])

    nc.compile()
    return nc


def _get_program():
    global _PROGRAM
    if _PROGRAM is None:
        _PROGRAM = _build_program()
    return _PROGRAM


def _aug_x(x_b, n):
    """xaug [7, n]: rows x0,x1,x2, -x2hi, -x2lo, 1, 1 (fp16)."""
    f16 = np.float16
    xa = np.ones((7, n), f16)
    x16 = x_b.astype(f16)
    xa[0:3] = x16.T
    x2 = (x16.astype(np.float32) ** 2).sum(1)
    hi = x2.astype(f16)
    xa[3] = -hi
    xa[4] = -(x2 - hi.astype(np.float32)).astype(f16)
    return xa


def _prep_core_inputs(fine_b, coarse_b, gt_b):
    f16 = np.float16
    g16 = gt_b.astype(f16)  # [3, M]
    yaug = np.ones((7, M), f16)
    yaug[0:3] = (2.0 * g16.astype(np.float32)).astype(f16)
    y2 = (g16.astype(np.float32) ** 2).sum(0)
    hi = y2.astype(f16)
    yaug[5] = -hi
    yaug[6] = -(y2 - hi.astype(np.float32)).astype(f16)
    return {
        "xaug_f": _aug_x(fine_b, NF),
        "xaug_c": _aug_x(coarse_b, NC_),
        "yaug": yaug,
    }


def kernel(coarse, fine, gt, alpha):
    global LAST_RESULTS
    from concourse import bass_utils

    coarse = np.asarray(coarse, np.float32)
    fine = np.asarray(fine, np.float32)
    gt = np.asarray(gt, np.float32)
    alpha = np.float32(np.asarray(alpha))

    nc = _get_program()
    in_maps = [_prep_core_inputs(fine[b], coarse[b], gt[b]) for b in range(B)]
    res = bass_utils.run_bass_kernel_spmd(
        nc, in_maps, core_ids=list(range(B)), trace=PROFILE
    )
    LAST_RESULTS = res
    per = np.stack([r["out"][0] for r in res.results]).astype(np.float64)  # [B, 8]
    # outputs hold NEGATED sums: 0=fine row, 2=coarse row,
    # 3/4=fine col (X part, tail), 6/7=coarse col (X part, tail)
    lf = np.float32((-per[:, 0] / NF - (per[:, 3] + per[:, 4]) / M).mean())
    lc = np.float32((-per[:, 2] / NC_ - (per[:, 6] + per[:, 7]) / M).mean())
    loss = np.float32(lc + np.float32(alpha) * lf)
    return (loss, lc, lf)


if __name__ == "__main__":
    rng = np.random.default_rng(0)
    out = kernel(
        coarse=rng.standard_normal((B, NC_, 3)).astype(np.float32),
        fine=rng.standard_normal((B, NF, 3)).astype(np.float32),
        gt=rng.standard_normal((B, 3, M)).astype(np.float32),
        alpha=np.float32(1.0),
    )
    print(out)


# revision 48
# speedup vs baseline: 1.0216x; 1.0216x over previous
"""Chamfer-distance loss kernel for Trainium2 (8 NeuronCores, SPMD).

Problem: loss = chamfer(coarse, gt_pts) + alpha * chamfer(fine, gt_pts)
  coarse [8,1024,3], fine [8,8192,3], gt [8,3,8192] (channel-first), alpha scalar.
  chamfer(x,y) = mean_n min_m d(n,m) + mean_m min_n d(n,m), d = squared L2.

Sharding: data-parallel over batch - one batch element per NeuronCore.

Per-core pipeline (negated distances, DMA-transposed column path):
  - The PE produces NEGATED distances 128x512 at a time as a K=7 fp16 matmul:
      lhsT rows {x0,x1,x2, -x2hi,-x2lo, 1,1}
      rhs  rows {2y0,2y1,2y2, 1,1, -y2hi,-y2lo}
    so PSUM = -d in fp32 (|x|^2, |y|^2 at ~fp32 precision via fp16 hi/lo
    splits of presummed norms). All mins are maxes; the host negates at the
    end. Negation lets partition collapses use gpsimd partition_all_reduce
    (max) on the otherwise-idle Pool engine (supports add/max/absmax only).
  - Every (tile, m-half) produces S fp16 [128, 4096] via a cast pass split
    between ScalarE (ACTF.Copy) and VectorE (tensor_copy) - the only two
    engines with PSUM access; they are jointly the drain bottleneck.
  - Row direction: one tensor_scalar per (tile, half) at fp16 4x mode
    (op0=max vs -60000 = identity, op1=max into accum_out rowW[:, i, mh]).
  - Col direction:
      * fine m-lo half + head of m-hi: S is DMA-TRANSPOSED (XBAR, idle DMA
        engines) into TG[m_p, j, slot, n] for groups of GT=4 tiles (TG
        double-buffered); per (group, j) one tensor_scalar 4x accum-max over
        free=[4,128] yields uncontaminated per-m maxes at ~0.39 ns/elem
        instead of 0.52 (tensor_tensor 2x). Group flushes are emitted a few
        tiles late to give the DMA runway (engines execute in order).
      * fine m-hi tail + coarse: fp16 2x tensor_tensor running-max into acc
        tiles; partition collapse via Pool partition_all_reduce(max).
  - Scalar totals come from free-axis tensor_reduce(add) + Pool
    partition_all_reduce(add); no PE transposes or ones-matmuls.
  - HW-verified constraints: gpsimd/Pool tensor_tensor/tensor_scalar and DMA
    accum are ISA-rejected on TRN2; matmul psum out must be fp32; DVE 4x
    requires all-SBUF operands (arranged by the transposed-S path);
    partition_all_reduce and XBAR DMA transpose verified correct on HW.

Host does only O(N) prep and the final scalar arithmetic.
"""

import sys

sys.path.insert(0, "/opt/trn_rl_repo")

import numpy as np

B = 8
NF = 8192  # fine points
NC_ = 1024  # coarse points
M = 8192  # gt points
MH = M // 2  # m-half size
NJ = MH // 128  # 32 j-blocks per half

# --- tuning knobs ----------------------------------------------------------
GT = 4  # X-route group size (tiles per transposed group)
TG_BUFS = 2  # transposed-group buffer depth
CAST_DVE_MOD = 4  # every CAST_DVE_MOD-th cast group goes to VectorE
FLUSH_DELAY = 2  # tiles of DMA runway before a group flush
NJ2 = 18  # j-blocks of the m-hi half also X-routed (rest: tt route)

# --- module-level program cache -------------------------------------------
_PROGRAM = None
PROFILE = False
LAST_RESULTS = None


def _build_program():
    from concourse import bacc, bass, tile, bass_isa
    import concourse.mybir as mybir

    f16, f32 = mybir.dt.float16, mybir.dt.float32
    AL = mybir.AluOpType
    ACTF = mybir.ActivationFunctionType
    RED = bass_isa.ReduceOp

    nc = bacc.Bacc("TRN2", target_bir_lowering=False, debug=False, num_devices=B)

    xaug_f = nc.dram_tensor("xaug_f", [7, NF], f16, kind="ExternalInput")
    xaug_c = nc.dram_tensor("xaug_c", [7, NC_], f16, kind="ExternalInput")
    yaug_d = nc.dram_tensor("yaug", [7, M], f16, kind="ExternalInput")
    out_d = nc.dram_tensor("out", [1, 8], f32, kind="ExternalOutput")

    n_fine_tiles = NF // 128  # 64
    n_coarse_tiles = NC_ // 128  # 8
    assert n_fine_tiles % GT == 0

    cast_counter = [0]

    with tile.TileContext(nc) as tc:
        with (
            tc.tile_pool(name="const", bufs=1) as cpool,
            tc.tile_pool(name="s", bufs=3) as spool,
            tc.tile_pool(name="scr", bufs=2) as scrpool,
            tc.tile_pool(name="tg", bufs=TG_BUFS) as tgpool,
            tc.tile_pool(name="tg2", bufs=TG_BUFS) as tg2pool,
            tc.tile_pool(name="ct", bufs=2) as ctpool,
            tc.tile_pool(name="rr", bufs=1) as rrpool,
            tc.tile_pool(name="fin", bufs=1) as fpool,
            tc.tile_pool(name="ps", bufs=2, space=bass.MemorySpace.PSUM) as pspool,
        ):
            warm = cpool.tile([7, 512], f16)
            nc.gpsimd.memset(warm[:], 1.0)
            wsink = cpool.tile([1, 1], f32)
            for _w in range(6):
                wps = pspool.tile([128, 512], f32, name=f"wps{_w}", tag="psB")
                nc.tensor.matmul(wps[:], lhsT=warm[:, 0:128], rhs=warm[:, 0:512],
                                 start=True, stop=True)
                nc.vector.tensor_copy(wsink[:], wps[0:1, 0:1])
            Xf = cpool.tile([7, NF], f16)
            nc.sync.dma_start(Xf[:], xaug_f.ap())
            Xc = cpool.tile([7, NC_], f16)
            nc.sync.dma_start(Xc[:], xaug_c.ap())
            Y = cpool.tile([7, M], f16)
            nc.sync.dma_start(Y[:], yaug_d.ap())

            outb = cpool.tile([1, 8], f32)

            MT = MH - NJ2 * 128  # m-hi tail width kept on the tt route
            NJA = NJ + NJ2  # X-routed j-blocks per tile
            # per-family m-hi-tail col accumulators (tt route)
            accF = cpool.tile([128, MT], f16, name="accF") if MT else None
            accC = cpool.tile([128, MT], f16, name="accC") if MT else None
            rowWf = cpool.tile([128, n_fine_tiles, 2], f32)
            rowWc = cpool.tile([128, n_coarse_tiles, 2], f32)
            colWF = cpool.tile([128, NJA], f32)  # fine X-route per-m col maxes
            colWC = cpool.tile([128, NJA], f32)  # coarse X-route col maxes

            def make_S(Xa, i, mh):
                """matmuls + cast -> S fp16 [128, MH] (= -d), m-half mh.
                Asymmetric psum slots: one 2048 (single-buffered, always Act)
                + two 1024 (double-buffered ring; DVE-cast eligible) so a
                slow DVE cast never blocks the PE refill rotation."""
                S = spool.tile([128, MH], f16, tag="S")
                for off, width, ptag, pbufs in (
                    (0, 2048, "psA", 1),
                    (2048, 1024, "psB", 2),
                    (3072, 1024, "psB", 2),
                ):
                    ps = pspool.tile([128, width], f32, tag=ptag, bufs=pbufs)
                    for j in range(width // 512):
                        mlo = mh * MH + off + j * 512
                        nc.tensor.matmul(
                            ps[:, j * 512 : (j + 1) * 512],
                            lhsT=Xa[:, i * 128 : (i + 1) * 128],
                            rhs=Y[:, mlo : mlo + 512],
                            start=True,
                            stop=True,
                        )
                    dve = False
                    if width == 1024:
                        c = cast_counter[0]
                        cast_counter[0] += 1
                        dve = c % CAST_DVE_MOD == 3 or c < 1
                    if dve:
                        nc.vector.tensor_copy(S[:, off : off + width], ps[:])
                    else:
                        nc.scalar.activation(
                            S[:, off : off + width],
                            ps[:],
                            ACTF.Copy,
                            bias=0.0,
                            scale=1.0,
                        )
                return S

            def row_max(S, rowW, i, mh):
                scr = scrpool.tile([128, MH], f16, tag="scr")
                nc.vector.tensor_scalar(
                    out=scr[:],
                    in0=S[:],
                    scalar1=-60000.0,
                    scalar2=None,
                    op0=AL.max,
                    op1=AL.max,
                    accum_out=rowW[:, i, mh : mh + 1],
                )

            def scalar_out(red, oidx):
                """red [128, 1] f32 -> outb[0, oidx] via Pool partition sum."""
                pr = rrpool.tile([128, 1], f32, tag="par1")
                nc.gpsimd.partition_all_reduce(pr[:], red[:], channels=128,
                                               reduce_op=RED.add)
                nc.vector.tensor_copy(outb[0:1, oidx : oidx + 1], pr[0:1, 0:1])

            def collapse_sum(acc, oidx):
                """Pool-collapse acc [128, W] f16 over partitions (in 2048-wide
                chunks), then sum over m into outb[0, oidx]."""
                w = acc.shape[1]
                nchunks = (w + 2047) // 2048
                redc = fpool.tile([128, nchunks], f32, tag=f"redc{oidx}")
                for k in range(nchunks):
                    cw = min(2048, w - k * 2048)
                    rr = rrpool.tile([128, 2048], f32, tag="rr")
                    nc.gpsimd.partition_all_reduce(
                        rr[:, 0:cw], acc[:, k * 2048 : k * 2048 + cw],
                        channels=128, reduce_op=RED.max,
                    )
                    nc.vector.tensor_reduce(
                        out=redc[:, k : k + 1], in_=rr[:, 0:cw],
                        axis=mybir.AxisListType.X, op=AL.add,
                    )
                red = fpool.tile([128, 1], f32, tag=f"cred{oidx}")
                nc.vector.tensor_reduce(
                    out=red[:], in_=redc[:], axis=mybir.AxisListType.X, op=AL.add
                )
                nc.vector.tensor_copy(outb[0:1, oidx : oidx + 1], red[0:1, 0:1])

            # ---- unified tile stream: coarse first, then fine; every tile
            # X-routes m-lo fully and the head of m-hi; tails go through
            # tensor_tensor accumulators ----
            first_group = {"f": True, "c": True}

            def flush_group(TG, TG2, colW, fam, nS=GT):
                if first_group[fam]:
                    tgt = colW
                else:
                    tgt = ctpool.tile([128, NJA], f32, tag="colT")
                for j in range(NJ):
                    scr2 = scrpool.tile([128, GT * 128], f16, tag="scrj")
                    nc.vector.tensor_scalar(
                        out=scr2[:, 0 : nS * 128],
                        in0=TG[:, j, 0:nS, :],
                        scalar1=-60000.0,
                        scalar2=None,
                        op0=AL.max,
                        op1=AL.max,
                        accum_out=tgt[:, j : j + 1],
                    )
                for j in range(NJ2):
                    scr2 = scrpool.tile([128, GT * 128], f16, tag="scrj")
                    nc.vector.tensor_scalar(
                        out=scr2[:, 0 : nS * 128],
                        in0=TG2[:, j, 0:nS, :],
                        scalar1=-60000.0,
                        scalar2=None,
                        op0=AL.max,
                        op1=AL.max,
                        accum_out=tgt[:, NJ + j : NJ + j + 1],
                    )
                if tgt is not colW:
                    nc.vector.tensor_tensor(
                        out=colW[:], in0=colW[:], in1=tgt[:], op=AL.max
                    )
                first_group[fam] = False

            tiles = [("c", i) for i in range(n_coarse_tiles)] + [
                ("f", i) for i in range(n_fine_tiles)
            ]
            n_groups_full = len(tiles) // GT
            group_sizes = [GT] * n_groups_full
            assert sum(group_sizes) == len(tiles)
            gidx = [0]
            TG = TG2 = None
            in_group = 0
            step = 0
            pending = []  # (due_step, flush args) delayed for DMA runway
            for fam, i in tiles:
                Xa = Xf if fam == "f" else Xc
                rowW = rowWf if fam == "f" else rowWc
                acc = accF if fam == "f" else accC
                colW = colWF if fam == "f" else colWC
                # m-lo half: X route
                S = make_S(Xa, i, 0)
                row_max(S, rowW, i, 0)
                if in_group == 0:
                    TG = tgpool.tile([128, NJ, GT, 128], f16, tag="TG")
                    TG2 = tg2pool.tile(
                        [128, NJ2, GT, 128], f16, name="TG2", tag="TG2"
                    )
                nc.sync.dma_start(TG[:, :, in_group, :], S[:], transpose=True)
                # m-hi half: head X-routed, tail tt-routed
                S = make_S(Xa, i, 1)
                row_max(S, rowW, i, 1)
                nc.sync.dma_start(
                    TG2[:, :, in_group, :], S[:, 0 : NJ2 * 128], transpose=True
                )
                if MT:
                    if i == 0:
                        nc.vector.tensor_copy(acc[:], S[:, NJ2 * 128 : MH])
                    else:
                        nc.vector.tensor_tensor(
                            out=acc[:],
                            in0=acc[:],
                            in1=S[:, NJ2 * 128 : MH],
                            op=AL.max,
                        )
                in_group += 1
                gsz = group_sizes[gidx[0]]
                if in_group == gsz:
                    pending.append(
                        (step + 2 * FLUSH_DELAY, (TG, TG2, colW, fam, gsz))
                    )
                    in_group = 0
                    gidx[0] += 1
                step += 1
                while pending and pending[0][0] <= step:
                    flush_group(*pending.pop(0)[1])
                # mid-stream coarse tail collapse (Pool engine, async)
                if MT and fam == "f" and i == 2:
                    collapse_sum(accC, 7)

            # ---- finals ----
            def row_total(rowW, nT, oidx):
                rmax = fpool.tile([128, nT], f32, tag=f"rmax{oidx}")
                nc.vector.tensor_reduce(
                    out=rmax[:], in_=rowW[:], axis=mybir.AxisListType.X, op=AL.max
                )
                rsum = fpool.tile([128, 1], f32, tag=f"rsum{oidx}")
                nc.vector.tensor_reduce(
                    out=rsum[:], in_=rmax[:], axis=mybir.AxisListType.X, op=AL.add
                )
                scalar_out(rsum, oidx)

            # colW-independent finals first: the trailing group flushes wait
            # on DMA transposes, so this work overlaps them
            if MT:
                collapse_sum(accF, 4)
            else:
                nc.vector.memset(outb[0:1, 4:5], 0.0)
            row_total(rowWf, n_fine_tiles, 0)
            row_total(rowWc, n_coarse_tiles, 2)
            for _, args in pending:
                flush_group(*args)

            # X-route col sums
            cxr = fpool.tile([128, 1], f32, tag="cxr")
            nc.vector.tensor_reduce(
                out=cxr[:], in_=colWF[:], axis=mybir.AxisListType.X, op=AL.add
            )
            scalar_out(cxr, 3)
            cxc = fpool.tile([128, 1], f32, tag="cxc")
            nc.vector.tensor_reduce(
                out=cxc[:], in_=colWC[:], axis=mybir.AxisListType.X, op=AL.add
            )
            scalar_out(cxc, 6)
            if not MT:
                nc.vector.memset(outb[0:1, 7:8], 0.0)

            nc.vector.memset(outb[0:1, 1:2], 0.0)
            nc.vector.memset(outb[0:1, 5:6], 0.0)
            nc.sync.dma_start(out_d.ap(), outb[:])

    nc.compile()
    return nc


def _get_program():
    global _PROGRAM
    if _PROGRAM is None:
        _PROGRAM = _build_program()
    return _PROGRAM


def _aug_x(x_b, n):
    """xaug [7, n]: rows x0,x1,x2, -x2hi, -x2lo, 1, 1 (fp16)."""
    f16 = np.float16
    xa = np.ones((7, n), f16)
    x16 = x_b.astype(f16)
    xa[0:3] = x16.T
    x2 = (x16.astype(np.float32) ** 2).sum(1)
    hi = x2.astype(f16)
    xa[3] = -hi
    xa[4] = -(x2 - hi.astype(np.float32)).astype(f16)
    return xa


def _prep_core_inputs(fine_b, coarse_b, gt_b):
    f16 = np.float16
    g16 = gt_b.astype(f16)  # [3, M]
    yaug = np.ones((7, M), f16)
    yaug[0:3] = (2.0 * g16.astype(np.float32)).astype(f16)
    y2 = (g16.astype(np.float32) ** 2).sum(0)
    hi = y2.astype(f16)
    yaug[5] = -hi
    yaug[6] = -(y2 - hi.astype(np.float32)).astype(f16)
    return {
        "xaug_f": _aug_x(fine_b, NF),
        "xaug_c": _aug_x(coarse_b, NC_),
        "yaug": yaug,
    }


def kernel(coarse, fine, gt, alpha):
    global LAST_RESULTS
    from concourse import bass_utils

    coarse = np.asarray(coarse, np.float32)
    fine = np.asarray(fine, np.float32)
    gt = np.asarray(gt, np.float32)
    alpha = np.float32(np.asarray(alpha))

    nc = _get_program()
    in_maps = [_prep_core_inputs(fine[b], coarse[b], gt[b]) for b in range(B)]
    res = bass_utils.run_bass_kernel_spmd(
        nc, in_maps, core_ids=list(range(B)), trace=PROFILE
    )
    LAST_RESULTS = res
    per = np.stack([r["out"][0] for r in res.results]).astype(np.float64)  # [B, 8]
    # outputs hold NEGATED sums: 0=fine row, 2=coarse row,
    # 3/4=fine col (X part, tail), 6/7=coarse col (X part, tail)
    lf = np.float32((-per[:, 0] / NF - (per[:, 3] + per[:, 4]) / M).mean())
    lc = np.float32((-per[:, 2] / NC_ - (per[:, 6] + per[:, 7]) / M).mean())
    loss = np.float32(lc + np.float32(alpha) * lf)
    return (loss, lc, lf)


if __name__ == "__main__":
    rng = np.random.default_rng(0)
    out = kernel(
        coarse=rng.standard_normal((B, NC_, 3)).astype(np.float32),
        fine=rng.standard_normal((B, NF, 3)).astype(np.float32),
        gt=rng.standard_normal((B, 3, M)).astype(np.float32),
        alpha=np.float32(1.0),
    )
    print(out)


# revision 49
# speedup vs baseline: 1.0257x; 1.0040x over previous
"""Chamfer-distance loss kernel for Trainium2 (8 NeuronCores, SPMD).

Problem: loss = chamfer(coarse, gt_pts) + alpha * chamfer(fine, gt_pts)
  coarse [8,1024,3], fine [8,8192,3], gt [8,3,8192] (channel-first), alpha scalar.
  chamfer(x,y) = mean_n min_m d(n,m) + mean_m min_n d(n,m), d = squared L2.

Sharding: data-parallel over batch - one batch element per NeuronCore.

Per-core pipeline (negated distances, DMA-transposed column path):
  - The PE produces NEGATED distances 128x512 at a time as a K=7 fp16 matmul:
      lhsT rows {x0,x1,x2, -x2hi,-x2lo, 1,1}
      rhs  rows {2y0,2y1,2y2, 1,1, -y2hi,-y2lo}
    so PSUM = -d in fp32 (|x|^2, |y|^2 at ~fp32 precision via fp16 hi/lo
    splits of presummed norms). All mins are maxes; the host negates at the
    end. Negation lets partition collapses use gpsimd partition_all_reduce
    (max) on the otherwise-idle Pool engine (supports add/max/absmax only).
  - Every (tile, m-half) produces S fp16 [128, 4096] via a cast pass split
    between ScalarE (ACTF.Copy) and VectorE (tensor_copy) - the only two
    engines with PSUM access; they are jointly the drain bottleneck.
  - Row direction: one tensor_scalar per (tile, half) at fp16 4x mode
    (op0=max vs -60000 = identity, op1=max into accum_out rowW[:, i, mh]).
  - Col direction:
      * fine m-lo half + head of m-hi: S is DMA-TRANSPOSED (XBAR, idle DMA
        engines) into TG[m_p, j, slot, n] for groups of GT=4 tiles (TG
        double-buffered); per (group, j) one tensor_scalar 4x accum-max over
        free=[4,128] yields uncontaminated per-m maxes at ~0.39 ns/elem
        instead of 0.52 (tensor_tensor 2x). Group flushes are emitted a few
        tiles late to give the DMA runway (engines execute in order).
      * fine m-hi tail + coarse: fp16 2x tensor_tensor running-max into acc
        tiles; partition collapse via Pool partition_all_reduce(max).
  - Scalar totals come from free-axis tensor_reduce(add) + Pool
    partition_all_reduce(add); no PE transposes or ones-matmuls.
  - HW-verified constraints: gpsimd/Pool tensor_tensor/tensor_scalar and DMA
    accum are ISA-rejected on TRN2; matmul psum out must be fp32; DVE 4x
    requires all-SBUF operands (arranged by the transposed-S path);
    partition_all_reduce and XBAR DMA transpose verified correct on HW.

Host does only O(N) prep and the final scalar arithmetic.
"""

import sys

sys.path.insert(0, "/opt/trn_rl_repo")

import numpy as np

B = 8
NF = 8192  # fine points
NC_ = 1024  # coarse points
M = 8192  # gt points
MH = M // 2  # m-half size
NJ = MH // 128  # 32 j-blocks per half

# --- tuning knobs ----------------------------------------------------------
GT = 4  # X-route group size (tiles per transposed group)
TG_BUFS = 2  # transposed-group buffer depth
CAST_DVE_MOD = 4  # every CAST_DVE_MOD-th cast group goes to VectorE
FLUSH_DELAY = 2  # tiles of DMA runway before a group flush
NJ2 = 18  # j-blocks of the m-hi half also X-routed (rest: tt route)

# --- module-level program cache -------------------------------------------
_PROGRAM = None
PROFILE = False
LAST_RESULTS = None


def _build_program():
    from concourse import bacc, bass, tile, bass_isa
    import concourse.mybir as mybir

    f16, f32 = mybir.dt.float16, mybir.dt.float32
    AL = mybir.AluOpType
    ACTF = mybir.ActivationFunctionType
    RED = bass_isa.ReduceOp

    nc = bacc.Bacc("TRN2", target_bir_lowering=False, debug=False, num_devices=B)

    xaug_f = nc.dram_tensor("xaug_f", [7, NF], f16, kind="ExternalInput")
    xaug_c = nc.dram_tensor("xaug_c", [7, NC_], f16, kind="ExternalInput")
    yaug_d = nc.dram_tensor("yaug", [7, M], f16, kind="ExternalInput")
    out_d = nc.dram_tensor("out", [1, 8], f32, kind="ExternalOutput")

    n_fine_tiles = NF // 128  # 64
    n_coarse_tiles = NC_ // 128  # 8
    assert n_fine_tiles % GT == 0

    cast_counter = [0]

    with tile.TileContext(nc) as tc:
        with (
            tc.tile_pool(name="const", bufs=1) as cpool,
            tc.tile_pool(name="s", bufs=4) as spool,
            tc.tile_pool(name="scr", bufs=2) as scrpool,
            tc.tile_pool(name="tg", bufs=TG_BUFS) as tgpool,
            tc.tile_pool(name="tg2", bufs=TG_BUFS) as tg2pool,
            tc.tile_pool(name="ct", bufs=2) as ctpool,
            tc.tile_pool(name="rr", bufs=1) as rrpool,
            tc.tile_pool(name="fin", bufs=1) as fpool,
            tc.tile_pool(name="ps", bufs=2, space=bass.MemorySpace.PSUM) as pspool,
        ):
            warm = cpool.tile([7, 512], f16)
            nc.gpsimd.memset(warm[:], 1.0)
            wsink = cpool.tile([1, 1], f32)
            for _w in range(6):
                wps = pspool.tile([128, 512], f32, name=f"wps{_w}", tag="psB")
                nc.tensor.matmul(wps[:], lhsT=warm[:, 0:128], rhs=warm[:, 0:512],
                                 start=True, stop=True)
                nc.vector.tensor_copy(wsink[:], wps[0:1, 0:1])
            Xf = cpool.tile([7, NF], f16)
            nc.sync.dma_start(Xf[:], xaug_f.ap())
            Xc = cpool.tile([7, NC_], f16)
            nc.sync.dma_start(Xc[:], xaug_c.ap())
            Y = cpool.tile([7, M], f16)
            nc.sync.dma_start(Y[:], yaug_d.ap())

            outb = cpool.tile([1, 8], f32)

            MT = MH - NJ2 * 128  # m-hi tail width kept on the tt route
            NJA = NJ + NJ2  # X-routed j-blocks per tile
            # per-family m-hi-tail col accumulators (tt route)
            accF = cpool.tile([128, MT], f16, name="accF") if MT else None
            accC = cpool.tile([128, MT], f16, name="accC") if MT else None
            rowWf = cpool.tile([128, n_fine_tiles, 2], f32)
            rowWc = cpool.tile([128, n_coarse_tiles, 2], f32)
            colWF = cpool.tile([128, NJA], f32)  # fine X-route per-m col maxes
            colWC = cpool.tile([128, NJA], f32)  # coarse X-route col maxes

            def make_S(Xa, i, mh):
                """matmuls + cast -> S fp16 [128, MH] (= -d), m-half mh.
                Asymmetric psum slots: one 2048 (single-buffered, always Act)
                + two 1024 (double-buffered ring; DVE-cast eligible) so a
                slow DVE cast never blocks the PE refill rotation."""
                S = spool.tile([128, MH], f16, tag="S")
                for off, width, ptag, pbufs in (
                    (0, 2048, "psA", 1),
                    (2048, 1024, "psB", 2),
                    (3072, 1024, "psB", 2),
                ):
                    ps = pspool.tile([128, width], f32, tag=ptag, bufs=pbufs)
                    for j in range(width // 512):
                        mlo = mh * MH + off + j * 512
                        nc.tensor.matmul(
                            ps[:, j * 512 : (j + 1) * 512],
                            lhsT=Xa[:, i * 128 : (i + 1) * 128],
                            rhs=Y[:, mlo : mlo + 512],
                            start=True,
                            stop=True,
                        )
                    dve = False
                    if width == 1024:
                        c = cast_counter[0]
                        cast_counter[0] += 1
                        dve = c % CAST_DVE_MOD == 3 or c < 1
                    if dve:
                        nc.vector.tensor_copy(S[:, off : off + width], ps[:])
                    else:
                        nc.scalar.activation(
                            S[:, off : off + width],
                            ps[:],
                            ACTF.Copy,
                            bias=0.0,
                            scale=1.0,
                        )
                return S

            def row_max(S, rowW, i, mh):
                scr = scrpool.tile([128, MH], f16, tag="scr")
                nc.vector.tensor_scalar(
                    out=scr[:],
                    in0=S[:],
                    scalar1=-60000.0,
                    scalar2=None,
                    op0=AL.max,
                    op1=AL.max,
                    accum_out=rowW[:, i, mh : mh + 1],
                )

            def scalar_out(red, oidx):
                """red [128, 1] f32 -> outb[0, oidx] via Pool partition sum."""
                pr = rrpool.tile([128, 1], f32, tag="par1")
                nc.gpsimd.partition_all_reduce(pr[:], red[:], channels=128,
                                               reduce_op=RED.add)
                nc.vector.tensor_copy(outb[0:1, oidx : oidx + 1], pr[0:1, 0:1])

            def collapse_sum(acc, oidx):
                """Pool-collapse acc [128, W] f16 over partitions (in 2048-wide
                chunks), then sum over m into outb[0, oidx]."""
                w = acc.shape[1]
                nchunks = (w + 2047) // 2048
                redc = fpool.tile([128, nchunks], f32, tag=f"redc{oidx}")
                for k in range(nchunks):
                    cw = min(2048, w - k * 2048)
                    rr = rrpool.tile([128, 2048], f32, tag="rr")
                    nc.gpsimd.partition_all_reduce(
                        rr[:, 0:cw], acc[:, k * 2048 : k * 2048 + cw],
                        channels=128, reduce_op=RED.max,
                    )
                    nc.vector.tensor_reduce(
                        out=redc[:, k : k + 1], in_=rr[:, 0:cw],
                        axis=mybir.AxisListType.X, op=AL.add,
                    )
                red = fpool.tile([128, 1], f32, tag=f"cred{oidx}")
                nc.vector.tensor_reduce(
                    out=red[:], in_=redc[:], axis=mybir.AxisListType.X, op=AL.add
                )
                nc.vector.tensor_copy(outb[0:1, oidx : oidx + 1], red[0:1, 0:1])

            # ---- unified tile stream: coarse first, then fine; every tile
            # X-routes m-lo fully and the head of m-hi; tails go through
            # tensor_tensor accumulators ----
            first_group = {"f": True, "c": True}

            def flush_group(TG, TG2, colW, fam, nS=GT):
                if first_group[fam]:
                    tgt = colW
                else:
                    tgt = ctpool.tile([128, NJA], f32, tag="colT")
                for j in range(NJ):
                    scr2 = scrpool.tile([128, GT * 128], f16, tag="scrj")
                    nc.vector.tensor_scalar(
                        out=scr2[:, 0 : nS * 128],
                        in0=TG[:, j, 0:nS, :],
                        scalar1=-60000.0,
                        scalar2=None,
                        op0=AL.max,
                        op1=AL.max,
                        accum_out=tgt[:, j : j + 1],
                    )
                for j in range(NJ2):
                    scr2 = scrpool.tile([128, GT * 128], f16, tag="scrj")
                    nc.vector.tensor_scalar(
                        out=scr2[:, 0 : nS * 128],
                        in0=TG2[:, j, 0:nS, :],
                        scalar1=-60000.0,
                        scalar2=None,
                        op0=AL.max,
                        op1=AL.max,
                        accum_out=tgt[:, NJ + j : NJ + j + 1],
                    )
                if tgt is not colW:
                    nc.vector.tensor_tensor(
                        out=colW[:], in0=colW[:], in1=tgt[:], op=AL.max
                    )
                first_group[fam] = False

            tiles = [("c", i) for i in range(n_coarse_tiles)] + [
                ("f", i) for i in range(n_fine_tiles)
            ]
            n_groups_full = len(tiles) // GT
            group_sizes = [GT] * n_groups_full
            assert sum(group_sizes) == len(tiles)
            gidx = [0]
            TG = TG2 = None
            in_group = 0
            step = 0
            pending = []  # (due_step, flush args) delayed for DMA runway
            for fam, i in tiles:
                Xa = Xf if fam == "f" else Xc
                rowW = rowWf if fam == "f" else rowWc
                acc = accF if fam == "f" else accC
                colW = colWF if fam == "f" else colWC
                # m-lo half: X route
                S = make_S(Xa, i, 0)
                row_max(S, rowW, i, 0)
                if in_group == 0:
                    TG = tgpool.tile([128, NJ, GT, 128], f16, tag="TG")
                    TG2 = tg2pool.tile(
                        [128, NJ2, GT, 128], f16, name="TG2", tag="TG2"
                    )
                nc.sync.dma_start(TG[:, :, in_group, :], S[:], transpose=True)
                # m-hi half: head X-routed, tail tt-routed
                S = make_S(Xa, i, 1)
                row_max(S, rowW, i, 1)
                nc.sync.dma_start(
                    TG2[:, :, in_group, :], S[:, 0 : NJ2 * 128], transpose=True
                )
                if MT:
                    if i == 0:
                        nc.vector.tensor_copy(acc[:], S[:, NJ2 * 128 : MH])
                    else:
                        nc.vector.tensor_tensor(
                            out=acc[:],
                            in0=acc[:],
                            in1=S[:, NJ2 * 128 : MH],
                            op=AL.max,
                        )
                in_group += 1
                gsz = group_sizes[gidx[0]]
                if in_group == gsz:
                    pending.append(
                        (step + 2 * FLUSH_DELAY, (TG, TG2, colW, fam, gsz))
                    )
                    in_group = 0
                    gidx[0] += 1
                step += 1
                while pending and pending[0][0] <= step:
                    flush_group(*pending.pop(0)[1])
                # mid-stream coarse tail collapse (Pool engine, async)
                if MT and fam == "f" and i == 2:
                    collapse_sum(accC, 7)

            # ---- finals ----
            def row_total(rowW, nT, oidx):
                rmax = fpool.tile([128, nT], f32, tag=f"rmax{oidx}")
                nc.vector.tensor_reduce(
                    out=rmax[:], in_=rowW[:], axis=mybir.AxisListType.X, op=AL.max
                )
                rsum = fpool.tile([128, 1], f32, tag=f"rsum{oidx}")
                nc.vector.tensor_reduce(
                    out=rsum[:], in_=rmax[:], axis=mybir.AxisListType.X, op=AL.add
                )
                scalar_out(rsum, oidx)

            # colW-independent finals first: the trailing group flushes wait
            # on DMA transposes, so this work overlaps them
            if MT:
                collapse_sum(accF, 4)
            else:
                nc.vector.memset(outb[0:1, 4:5], 0.0)
            row_total(rowWf, n_fine_tiles, 0)
            row_total(rowWc, n_coarse_tiles, 2)
            for _, args in pending:
                flush_group(*args)

            # X-route col sums
            cxr = fpool.tile([128, 1], f32, tag="cxr")
            nc.vector.tensor_reduce(
                out=cxr[:], in_=colWF[:], axis=mybir.AxisListType.X, op=AL.add
            )
            scalar_out(cxr, 3)
            cxc = fpool.tile([128, 1], f32, tag="cxc")
            nc.vector.tensor_reduce(
                out=cxc[:], in_=colWC[:], axis=mybir.AxisListType.X, op=AL.add
            )
            scalar_out(cxc, 6)
            if not MT:
                nc.vector.memset(outb[0:1, 7:8], 0.0)

            nc.vector.memset(outb[0:1, 1:2], 0.0)
            nc.vector.memset(outb[0:1, 5:6], 0.0)
            nc.sync.dma_start(out_d.ap(), outb[:])

    nc.compile()
    return nc


def _get_program():
    global _PROGRAM
    if _PROGRAM is None:
        _PROGRAM = _build_program()
    return _PROGRAM


def _aug_x(x_b, n):
    """xaug [7, n]: rows x0,x1,x2, -x2hi, -x2lo, 1, 1 (fp16)."""
    f16 = np.float16
    xa = np.ones((7, n), f16)
    x16 = x_b.astype(f16)
    xa[0:3] = x16.T
    x2 = (x16.astype(np.float32) ** 2).sum(1)
    hi = x2.astype(f16)
    xa[3] = -hi
    xa[4] = -(x2 - hi.astype(np.float32)).astype(f16)
    return xa


def _prep_core_inputs(fine_b, coarse_b, gt_b):
    f16 = np.float16
    g16 = gt_b.astype(f16)  # [3, M]
    yaug = np.ones((7, M), f16)
    yaug[0:3] = (2.0 * g16.astype(np.float32)).astype(f16)
    y2 = (g16.astype(np.float32) ** 2).sum(0)
    hi = y2.astype(f16)
    yaug[5] = -hi
    yaug[6] = -(y2 - hi.astype(np.float32)).astype(f16)
    return {
        "xaug_f": _aug_x(fine_b, NF),
        "xaug_c": _aug_x(coarse_b, NC_),
        "yaug": yaug,
    }


def kernel(coarse, fine, gt, alpha):
    global LAST_RESULTS
    from concourse import bass_utils

    coarse = np.asarray(coarse, np.float32)
    fine = np.asarray(fine, np.float32)
    gt = np.asarray(gt, np.float32)
    alpha = np.float32(np.asarray(alpha))

    nc = _get_program()
    in_maps = [_prep_core_inputs(fine[b], coarse[b], gt[b]) for b in range(B)]
    res = bass_utils.run_bass_kernel_spmd(
        nc, in_maps, core_ids=list(range(B)), trace=PROFILE
    )
    LAST_RESULTS = res
    per = np.stack([r["out"][0] for r in res.results]).astype(np.float64)  # [B, 8]
    # outputs hold NEGATED sums: 0=fine row, 2=coarse row,
    # 3/4=fine col (X part, tail), 6/7=coarse col (X part, tail)
    lf = np.float32((-per[:, 0] / NF - (per[:, 3] + per[:, 4]) / M).mean())
    lc = np.float32((-per[:, 2] / NC_ - (per[:, 6] + per[:, 7]) / M).mean())
    loss = np.float32(lc + np.float32(alpha) * lf)
    return (loss, lc, lf)


if __name__ == "__main__":
    rng = np.random.default_rng(0)
    out = kernel(
        coarse=rng.standard_normal((B, NC_, 3)).astype(np.float32),
        fine=rng.standard_normal((B, NF, 3)).astype(np.float32),
        gt=rng.standard_normal((B, 3, M)).astype(np.float32),
        alpha=np.float32(1.0),
    )
    print(out)


# revision 50
# speedup vs baseline: 1.0280x; 1.0022x over previous
"""Chamfer-distance loss kernel for Trainium2 (8 NeuronCores, SPMD).

Problem: loss = chamfer(coarse, gt_pts) + alpha * chamfer(fine, gt_pts)
  coarse [8,1024,3], fine [8,8192,3], gt [8,3,8192] (channel-first), alpha scalar.
  chamfer(x,y) = mean_n min_m d(n,m) + mean_m min_n d(n,m), d = squared L2.

Sharding: data-parallel over batch - one batch element per NeuronCore.

Per-core pipeline (negated distances, DMA-transposed column path):
  - The PE produces NEGATED distances 128x512 at a time as a K=7 fp16 matmul:
      lhsT rows {x0,x1,x2, -x2hi,-x2lo, 1,1}
      rhs  rows {2y0,2y1,2y2, 1,1, -y2hi,-y2lo}
    so PSUM = -d in fp32 (|x|^2, |y|^2 at ~fp32 precision via fp16 hi/lo
    splits of presummed norms). All mins are maxes; the host negates at the
    end. Negation lets partition collapses use gpsimd partition_all_reduce
    (max) on the otherwise-idle Pool engine (supports add/max/absmax only).
  - Every (tile, m-half) produces S fp16 [128, 4096] via a cast pass split
    between ScalarE (ACTF.Copy) and VectorE (tensor_copy) - the only two
    engines with PSUM access; they are jointly the drain bottleneck.
  - Row direction: one tensor_scalar per (tile, half) at fp16 4x mode
    (op0=max vs -60000 = identity, op1=max into accum_out rowW[:, i, mh]).
  - Col direction:
      * fine m-lo half + head of m-hi: S is DMA-TRANSPOSED (XBAR, idle DMA
        engines) into TG[m_p, j, slot, n] for groups of GT=4 tiles (TG
        double-buffered); per (group, j) one tensor_scalar 4x accum-max over
        free=[4,128] yields uncontaminated per-m maxes at ~0.39 ns/elem
        instead of 0.52 (tensor_tensor 2x). Group flushes are emitted a few
        tiles late to give the DMA runway (engines execute in order).
      * fine m-hi tail + coarse: fp16 2x tensor_tensor running-max into acc
        tiles; partition collapse via Pool partition_all_reduce(max).
  - Scalar totals come from free-axis tensor_reduce(add) + Pool
    partition_all_reduce(add); no PE transposes or ones-matmuls.
  - HW-verified constraints: gpsimd/Pool tensor_tensor/tensor_scalar and DMA
    accum are ISA-rejected on TRN2; matmul psum out must be fp32; DVE 4x
    requires all-SBUF operands (arranged by the transposed-S path);
    partition_all_reduce and XBAR DMA transpose verified correct on HW.

Host does only O(N) prep and the final scalar arithmetic.
"""

import sys

sys.path.insert(0, "/opt/trn_rl_repo")

import numpy as np

B = 8
NF = 8192  # fine points
NC_ = 1024  # coarse points
M = 8192  # gt points
MH = M // 2  # m-half size
NJ = MH // 128  # 32 j-blocks per half

# --- tuning knobs ----------------------------------------------------------
GT = 4  # X-route group size (tiles per transposed group)
TG_BUFS = 2  # transposed-group buffer depth
CAST_DVE_MOD = 4  # every CAST_DVE_MOD-th cast group goes to VectorE
FLUSH_DELAY = 2  # tiles of DMA runway before a group flush
NJ2 = 18  # j-blocks of the m-hi half also X-routed (rest: tt route)

# --- module-level program cache -------------------------------------------
_PROGRAM = None
PROFILE = False
LAST_RESULTS = None


def _build_program():
    from concourse import bacc, bass, tile, bass_isa
    import concourse.mybir as mybir

    f16, f32 = mybir.dt.float16, mybir.dt.float32
    AL = mybir.AluOpType
    ACTF = mybir.ActivationFunctionType
    RED = bass_isa.ReduceOp

    nc = bacc.Bacc("TRN2", target_bir_lowering=False, debug=False, num_devices=B)

    xaug_f = nc.dram_tensor("xaug_f", [7, NF], f16, kind="ExternalInput")
    xaug_c = nc.dram_tensor("xaug_c", [7, NC_], f16, kind="ExternalInput")
    yaug_d = nc.dram_tensor("yaug", [7, M], f16, kind="ExternalInput")
    out_d = nc.dram_tensor("out", [1, 8], f32, kind="ExternalOutput")

    n_fine_tiles = NF // 128  # 64
    n_coarse_tiles = NC_ // 128  # 8
    assert n_fine_tiles % GT == 0

    cast_counter = [0]

    with tile.TileContext(nc) as tc:
        with (
            tc.tile_pool(name="const", bufs=1) as cpool,
            tc.tile_pool(name="s", bufs=4) as spool,
            tc.tile_pool(name="scr", bufs=2) as scrpool,
            tc.tile_pool(name="tg", bufs=TG_BUFS) as tgpool,
            tc.tile_pool(name="tg2", bufs=TG_BUFS) as tg2pool,
            tc.tile_pool(name="ct", bufs=2) as ctpool,
            tc.tile_pool(name="rr", bufs=1) as rrpool,
            tc.tile_pool(name="fin", bufs=1) as fpool,
            tc.tile_pool(name="ps", bufs=2, space=bass.MemorySpace.PSUM) as pspool,
        ):
            warm = cpool.tile([7, 512], f16)
            nc.gpsimd.memset(warm[:], 1.0)
            wsink = cpool.tile([1, 1], f32)
            for _w in range(6):
                wps = pspool.tile([128, 512], f32, name=f"wps{_w}", tag="psB")
                nc.tensor.matmul(wps[:], lhsT=warm[:, 0:128], rhs=warm[:, 0:512],
                                 start=True, stop=True)
                nc.vector.tensor_copy(wsink[:], wps[0:1, 0:1])
            Xf = cpool.tile([7, NF], f16)
            nc.sync.dma_start(Xf[:], xaug_f.ap())
            Xc = cpool.tile([7, NC_], f16)
            nc.sync.dma_start(Xc[:], xaug_c.ap())
            Y = cpool.tile([7, M], f16)
            nc.sync.dma_start(Y[:], yaug_d.ap())

            outb = cpool.tile([1, 8], f32)

            MT = MH - NJ2 * 128  # m-hi tail width kept on the tt route
            NJA = NJ + NJ2  # X-routed j-blocks per tile
            # per-family m-hi-tail col accumulators (tt route)
            accF = cpool.tile([128, MT], f16, name="accF") if MT else None
            accC = cpool.tile([128, MT], f16, name="accC") if MT else None
            rowWf = cpool.tile([128, n_fine_tiles, 4], f32)
            rowWc = cpool.tile([128, n_coarse_tiles, 4], f32)
            nc.vector.memset(rowWf[:], -60000.0)
            nc.vector.memset(rowWc[:], -60000.0)
            colWF = cpool.tile([128, NJA], f32)  # fine X-route per-m col maxes
            colWC = cpool.tile([128, NJA], f32)  # coarse X-route col maxes

            def make_S(Xa, i, mh, rowW):
                """matmuls + cast -> S fp16 [128, MH] (= -d), m-half mh.
                Asymmetric psum slots: one 2048 (single-buffered, always Act)
                + two 1024 (double-buffered ring; DVE-cast eligible) so a
                slow DVE cast never blocks the PE refill rotation. A DVE cast
                on the last 1024 runs as tensor_scalar(max) whose accum_out
                IS that span's row-max for free; returns the remaining row
                span for row_max."""
                S = spool.tile([128, MH], f16, tag="S")
                row_hi = MH
                for off, width, ptag, pbufs in (
                    (0, 2048, "psA", 1),
                    (2048, 1024, "psB", 2),
                    (3072, 1024, "psB", 2),
                ):
                    ps = pspool.tile([128, width], f32, tag=ptag, bufs=pbufs)
                    for j in range(width // 512):
                        mlo = mh * MH + off + j * 512
                        nc.tensor.matmul(
                            ps[:, j * 512 : (j + 1) * 512],
                            lhsT=Xa[:, i * 128 : (i + 1) * 128],
                            rhs=Y[:, mlo : mlo + 512],
                            start=True,
                            stop=True,
                        )
                    dve = False
                    if width == 1024:
                        c = cast_counter[0]
                        cast_counter[0] += 1
                        dve = c % CAST_DVE_MOD == 3 or c < 1
                    if dve and off == 3072:
                        nc.vector.tensor_scalar(
                            out=S[:, off : off + width],
                            in0=ps[:],
                            scalar1=-60000.0,
                            scalar2=None,
                            op0=AL.max,
                            op1=AL.max,
                            accum_out=rowW[:, i, mh * 2 + 1 : mh * 2 + 2],
                        )
                        row_hi = off
                    elif dve:
                        nc.vector.tensor_copy(S[:, off : off + width], ps[:])
                    else:
                        nc.scalar.activation(
                            S[:, off : off + width],
                            ps[:],
                            ACTF.Copy,
                            bias=0.0,
                            scale=1.0,
                        )
                return S, row_hi

            def row_max(S, row_hi, rowW, i, mh):
                scr = scrpool.tile([128, MH], f16, tag="scr")
                nc.vector.tensor_scalar(
                    out=scr[:, 0:row_hi],
                    in0=S[:, 0:row_hi],
                    scalar1=-60000.0,
                    scalar2=None,
                    op0=AL.max,
                    op1=AL.max,
                    accum_out=rowW[:, i, mh * 2 : mh * 2 + 1],
                )

            def scalar_out(red, oidx):
                """red [128, 1] f32 -> outb[0, oidx] via Pool partition sum."""
                pr = rrpool.tile([128, 1], f32, tag="par1")
                nc.gpsimd.partition_all_reduce(pr[:], red[:], channels=128,
                                               reduce_op=RED.add)
                nc.vector.tensor_copy(outb[0:1, oidx : oidx + 1], pr[0:1, 0:1])

            def collapse_sum(acc, oidx):
                """Pool-collapse acc [128, W] f16 over partitions (in 2048-wide
                chunks), then sum over m into outb[0, oidx]."""
                w = acc.shape[1]
                nchunks = (w + 2047) // 2048
                redc = fpool.tile([128, nchunks], f32, tag=f"redc{oidx}")
                for k in range(nchunks):
                    cw = min(2048, w - k * 2048)
                    rr = rrpool.tile([128, 2048], f32, tag="rr")
                    nc.gpsimd.partition_all_reduce(
                        rr[:, 0:cw], acc[:, k * 2048 : k * 2048 + cw],
                        channels=128, reduce_op=RED.max,
                    )
                    nc.vector.tensor_reduce(
                        out=redc[:, k : k + 1], in_=rr[:, 0:cw],
                        axis=mybir.AxisListType.X, op=AL.add,
                    )
                red = fpool.tile([128, 1], f32, tag=f"cred{oidx}")
                nc.vector.tensor_reduce(
                    out=red[:], in_=redc[:], axis=mybir.AxisListType.X, op=AL.add
                )
                nc.vector.tensor_copy(outb[0:1, oidx : oidx + 1], red[0:1, 0:1])

            # ---- unified tile stream: coarse first, then fine; every tile
            # X-routes m-lo fully and the head of m-hi; tails go through
            # tensor_tensor accumulators ----
            first_group = {"f": True, "c": True}

            def flush_group(TG, TG2, colW, fam, nS=GT):
                if first_group[fam]:
                    tgt = colW
                else:
                    tgt = ctpool.tile([128, NJA], f32, tag="colT")
                for j in range(NJ):
                    scr2 = scrpool.tile([128, GT * 128], f16, tag="scrj")
                    nc.vector.tensor_scalar(
                        out=scr2[:, 0 : nS * 128],
                        in0=TG[:, j, 0:nS, :],
                        scalar1=-60000.0,
                        scalar2=None,
                        op0=AL.max,
                        op1=AL.max,
                        accum_out=tgt[:, j : j + 1],
                    )
                for j in range(NJ2):
                    scr2 = scrpool.tile([128, GT * 128], f16, tag="scrj")
                    nc.vector.tensor_scalar(
                        out=scr2[:, 0 : nS * 128],
                        in0=TG2[:, j, 0:nS, :],
                        scalar1=-60000.0,
                        scalar2=None,
                        op0=AL.max,
                        op1=AL.max,
                        accum_out=tgt[:, NJ + j : NJ + j + 1],
                    )
                if tgt is not colW:
                    nc.vector.tensor_tensor(
                        out=colW[:], in0=colW[:], in1=tgt[:], op=AL.max
                    )
                first_group[fam] = False

            tiles = [("c", i) for i in range(n_coarse_tiles)] + [
                ("f", i) for i in range(n_fine_tiles)
            ]
            n_groups_full = len(tiles) // GT
            group_sizes = [GT] * n_groups_full
            assert sum(group_sizes) == len(tiles)
            gidx = [0]
            TG = TG2 = None
            in_group = 0
            step = 0
            pending = []  # (due_step, flush args) delayed for DMA runway
            for fam, i in tiles:
                Xa = Xf if fam == "f" else Xc
                rowW = rowWf if fam == "f" else rowWc
                acc = accF if fam == "f" else accC
                colW = colWF if fam == "f" else colWC
                # m-lo half: X route
                S, rh = make_S(Xa, i, 0, rowW)
                row_max(S, rh, rowW, i, 0)
                if in_group == 0:
                    TG = tgpool.tile([128, NJ, GT, 128], f16, tag="TG")
                    TG2 = tg2pool.tile(
                        [128, NJ2, GT, 128], f16, name="TG2", tag="TG2"
                    )
                nc.sync.dma_start(TG[:, :, in_group, :], S[:], transpose=True)
                # m-hi half: head X-routed, tail tt-routed
                S, rh = make_S(Xa, i, 1, rowW)
                row_max(S, rh, rowW, i, 1)
                nc.sync.dma_start(
                    TG2[:, :, in_group, :], S[:, 0 : NJ2 * 128], transpose=True
                )
                if MT:
                    if i == 0:
                        nc.vector.tensor_copy(acc[:], S[:, NJ2 * 128 : MH])
                    else:
                        nc.vector.tensor_tensor(
                            out=acc[:],
                            in0=acc[:],
                            in1=S[:, NJ2 * 128 : MH],
                            op=AL.max,
                        )
                in_group += 1
                gsz = group_sizes[gidx[0]]
                if in_group == gsz:
                    pending.append(
                        (step + 2 * FLUSH_DELAY, (TG, TG2, colW, fam, gsz))
                    )
                    in_group = 0
                    gidx[0] += 1
                step += 1
                while pending and pending[0][0] <= step:
                    flush_group(*pending.pop(0)[1])
                # mid-stream coarse tail collapse (Pool engine, async)
                if MT and fam == "f" and i == 2:
                    collapse_sum(accC, 7)

            # ---- finals ----
            def row_total(rowW, nT, oidx):
                rmax = fpool.tile([128, nT], f32, tag=f"rmax{oidx}")
                nc.vector.tensor_reduce(
                    out=rmax[:], in_=rowW[:], axis=mybir.AxisListType.X, op=AL.max
                )
                rsum = fpool.tile([128, 1], f32, tag=f"rsum{oidx}")
                nc.vector.tensor_reduce(
                    out=rsum[:], in_=rmax[:], axis=mybir.AxisListType.X, op=AL.add
                )
                scalar_out(rsum, oidx)

            # colW-independent finals first: the trailing group flushes wait
            # on DMA transposes, so this work overlaps them
            if MT:
                collapse_sum(accF, 4)
            else:
                nc.vector.memset(outb[0:1, 4:5], 0.0)
            row_total(rowWf, n_fine_tiles, 0)
            row_total(rowWc, n_coarse_tiles, 2)
            for _, args in pending:
                flush_group(*args)

            # X-route col sums
            cxr = fpool.tile([128, 1], f32, tag="cxr")
            nc.vector.tensor_reduce(
                out=cxr[:], in_=colWF[:], axis=mybir.AxisListType.X, op=AL.add
            )
            scalar_out(cxr, 3)
            cxc = fpool.tile([128, 1], f32, tag="cxc")
            nc.vector.tensor_reduce(
                out=cxc[:], in_=colWC[:], axis=mybir.AxisListType.X, op=AL.add
            )
            scalar_out(cxc, 6)
            if not MT:
                nc.vector.memset(outb[0:1, 7:8], 0.0)

            nc.vector.memset(outb[0:1, 1:2], 0.0)
            nc.vector.memset(outb[0:1, 5:6], 0.0)
            nc.sync.dma_start(out_d.ap(), outb[:])

    nc.compile()
    return nc


def _get_program():
    global _PROGRAM
    if _PROGRAM is None:
        _PROGRAM = _build_program()
    return _PROGRAM


def _aug_x(x_b, n):
    """xaug [7, n]: rows x0,x1,x2, -x2hi, -x2lo, 1, 1 (fp16)."""
    f16 = np.float16
    xa = np.ones((7, n), f16)
    x16 = x_b.astype(f16)
    xa[0:3] = x16.T
    x2 = (x16.astype(np.float32) ** 2).sum(1)
    hi = x2.astype(f16)
    xa[3] = -hi
    xa[4] = -(x2 - hi.astype(np.float32)).astype(f16)
    return xa


def _prep_core_inputs(fine_b, coarse_b, gt_b):
    f16 = np.float16
    g16 = gt_b.astype(f16)  # [3, M]
    yaug = np.ones((7, M), f16)
    yaug[0:3] = (2.0 * g16.astype(np.float32)).astype(f16)
    y2 = (g16.astype(np.float32) ** 2).sum(0)
    hi = y2.astype(f16)
    yaug[5] = -hi
    yaug[6] = -(y2 - hi.astype(np.float32)).astype(f16)
    return {
        "xaug_f": _aug_x(fine_b, NF),
        "xaug_c": _aug_x(coarse_b, NC_),
        "yaug": yaug,
    }


def kernel(coarse, fine, gt, alpha):
    global LAST_RESULTS
    from concourse import bass_utils

    coarse = np.asarray(coarse, np.float32)
    fine = np.asarray(fine, np.float32)
    gt = np.asarray(gt, np.float32)
    alpha = np.float32(np.asarray(alpha))

    nc = _get_program()
    in_maps = [_prep_core_inputs(fine[b], coarse[b], gt[b]) for b in range(B)]
    res = bass_utils.run_bass_kernel_spmd(
        nc, in_maps, core_ids=list(range(B)), trace=PROFILE
    )
    LAST_RESULTS = res
    per = np.stack([r["out"][0] for r in res.results]).astype(np.float64)  # [B, 8]
    # outputs hold NEGATED sums: 0=fine row, 2=coarse row,
    # 3/4=fine col (X part, tail), 6/7=coarse col (X part, tail)
    lf = np.float32((-per[:, 0] / NF - (per[:, 3] + per[:, 4]) / M).mean())
    lc = np.float32((-per[:, 2] / NC_ - (per[:, 6] + per[:, 7]) / M).mean())
    loss = np.float32(lc + np.float32(alpha) * lf)
    return (loss, lc, lf)


if __name__ == "__main__":
    rng = np.random.default_rng(0)
    out = kernel(
        coarse=rng.standard_normal((B, NC_, 3)).astype(np.float32),
        fine=rng.standard_normal((B, NF, 3)).astype(np.float32),
        gt=rng.standard_normal((B, 3, M)).astype(np.float32),
        alpha=np.float32(1.0),
    )
    print(out)


# revision 51
# speedup vs baseline: 1.0328x; 1.0047x over previous
"""Chamfer-distance loss kernel for Trainium2 (8 NeuronCores, SPMD).

Problem: loss = chamfer(coarse, gt_pts) + alpha * chamfer(fine, gt_pts)
  coarse [8,1024,3], fine [8,8192,3], gt [8,3,8192] (channel-first), alpha scalar.
  chamfer(x,y) = mean_n min_m d(n,m) + mean_m min_n d(n,m), d = squared L2.

Sharding: data-parallel over batch - one batch element per NeuronCore.

Per-core pipeline (negated distances, DMA-transposed column path):
  - The PE produces NEGATED distances 128x512 at a time as a K=7 fp16 matmul:
      lhsT rows {x0,x1,x2, -x2hi,-x2lo, 1,1}
      rhs  rows {2y0,2y1,2y2, 1,1, -y2hi,-y2lo}
    so PSUM = -d in fp32 (|x|^2, |y|^2 at ~fp32 precision via fp16 hi/lo
    splits of presummed norms). All mins are maxes; the host negates at the
    end. Negation lets partition collapses use gpsimd partition_all_reduce
    (max) on the otherwise-idle Pool engine (supports add/max/absmax only).
  - Every (tile, m-half) produces S fp16 [128, 4096] via a cast pass split
    between ScalarE (ACTF.Copy) and VectorE (tensor_copy) - the only two
    engines with PSUM access; they are jointly the drain bottleneck.
  - Row direction: one tensor_scalar per (tile, half) at fp16 4x mode
    (op0=max vs -60000 = identity, op1=max into accum_out rowW[:, i, mh]).
  - Col direction:
      * fine m-lo half + head of m-hi: S is DMA-TRANSPOSED (XBAR, idle DMA
        engines) into TG[m_p, j, slot, n] for groups of GT=4 tiles (TG
        double-buffered); per (group, j) one tensor_scalar 4x accum-max over
        free=[4,128] yields uncontaminated per-m maxes at ~0.39 ns/elem
        instead of 0.52 (tensor_tensor 2x). Group flushes are emitted a few
        tiles late to give the DMA runway (engines execute in order).
      * fine m-hi tail + coarse: fp16 2x tensor_tensor running-max into acc
        tiles; partition collapse via Pool partition_all_reduce(max).
  - Scalar totals come from free-axis tensor_reduce(add) + Pool
    partition_all_reduce(add); no PE transposes or ones-matmuls.
  - HW-verified constraints: gpsimd/Pool tensor_tensor/tensor_scalar and DMA
    accum are ISA-rejected on TRN2; matmul psum out must be fp32; DVE 4x
    requires all-SBUF operands (arranged by the transposed-S path);
    partition_all_reduce and XBAR DMA transpose verified correct on HW.

Host does only O(N) prep and the final scalar arithmetic.
"""

import sys

sys.path.insert(0, "/opt/trn_rl_repo")

import numpy as np

B = 8
NF = 8192  # fine points
NC_ = 1024  # coarse points
M = 8192  # gt points
MH = M // 2  # m-half size
NJ = MH // 128  # 32 j-blocks per half

# --- tuning knobs ----------------------------------------------------------
GT = 4  # X-route group size (tiles per transposed group)
TG_BUFS = 2  # transposed-group buffer depth
CAST_DVE_MOD = 4  # every CAST_DVE_MOD-th cast group goes to VectorE
FLUSH_DELAY = 2  # tiles of DMA runway before a group flush
NJ2 = 18  # j-blocks of the m-hi half also X-routed (rest: tt route)

# --- module-level program cache -------------------------------------------
_PROGRAM = None
PROFILE = False
LAST_RESULTS = None


def _build_program():
    from concourse import bacc, bass, tile, bass_isa
    import concourse.mybir as mybir

    f16, f32 = mybir.dt.float16, mybir.dt.float32
    AL = mybir.AluOpType
    ACTF = mybir.ActivationFunctionType
    RED = bass_isa.ReduceOp

    nc = bacc.Bacc("TRN2", target_bir_lowering=False, debug=False, num_devices=B)

    xaug_f = nc.dram_tensor("xaug_f", [7, NF], f16, kind="ExternalInput")
    xaug_c = nc.dram_tensor("xaug_c", [7, NC_], f16, kind="ExternalInput")
    yaug_d = nc.dram_tensor("yaug", [7, M], f16, kind="ExternalInput")
    out_d = nc.dram_tensor("out", [1, 8], f32, kind="ExternalOutput")

    n_fine_tiles = NF // 128  # 64
    n_coarse_tiles = NC_ // 128  # 8
    assert n_fine_tiles % GT == 0

    cast_counter = [0]

    with tile.TileContext(nc) as tc:
        with (
            tc.tile_pool(name="const", bufs=1) as cpool,
            tc.tile_pool(name="s", bufs=4) as spool,
            tc.tile_pool(name="scr", bufs=2) as scrpool,
            tc.tile_pool(name="tg", bufs=TG_BUFS) as tgpool,
            tc.tile_pool(name="tg2", bufs=TG_BUFS) as tg2pool,
            tc.tile_pool(name="ct", bufs=2) as ctpool,
            tc.tile_pool(name="rr", bufs=1) as rrpool,
            tc.tile_pool(name="fin", bufs=1) as fpool,
            tc.tile_pool(name="ps", bufs=2, space=bass.MemorySpace.PSUM) as pspool,
        ):
            warm = cpool.tile([7, 512], f16)
            nc.gpsimd.memset(warm[:], 1.0)
            wsink = cpool.tile([1, 1], f32)
            for _w in range(6):
                wps = pspool.tile([128, 512], f32, name=f"wps{_w}", tag="psB")
                nc.tensor.matmul(wps[:], lhsT=warm[:, 0:128], rhs=warm[:, 0:512],
                                 start=True, stop=True)
                nc.vector.tensor_copy(wsink[:], wps[0:1, 0:1])
            Xf = cpool.tile([7, NF], f16)
            nc.sync.dma_start(Xf[:], xaug_f.ap())
            Xc = cpool.tile([7, NC_], f16)
            nc.sync.dma_start(Xc[:], xaug_c.ap())
            Y = cpool.tile([7, M], f16)
            nc.sync.dma_start(Y[:], yaug_d.ap())

            outb = cpool.tile([1, 8], f32)

            MT = MH - NJ2 * 128  # m-hi tail width kept on the tt route
            NJA = NJ + NJ2  # X-routed j-blocks per tile
            # per-family m-hi-tail col accumulators (tt route)
            accF = cpool.tile([128, MT], f16, name="accF") if MT else None
            accC = cpool.tile([128, MT], f16, name="accC") if MT else None
            rowWf = cpool.tile([128, n_fine_tiles, 4], f32)
            rowWc = cpool.tile([128, n_coarse_tiles, 4], f32)
            nc.vector.memset(rowWf[:], -60000.0)
            nc.vector.memset(rowWc[:], -60000.0)
            colWF = cpool.tile([128, NJA], f32)  # fine X-route per-m col maxes
            colWC = cpool.tile([128, NJA], f32)  # coarse X-route col maxes

            def make_S(Xa, i, mh, rowW):
                """matmuls + cast -> S fp16 [128, MH] (= -d), m-half mh.
                Asymmetric psum slots: one 2048 (single-buffered, always Act)
                + two 1024 (double-buffered ring; DVE-cast eligible) so a
                slow DVE cast never blocks the PE refill rotation. A DVE cast
                on the last 1024 runs as tensor_scalar(max) whose accum_out
                IS that span's row-max for free; returns the remaining row
                span for row_max."""
                S = spool.tile([128, MH], f16, tag="S")
                row_hi = MH
                for off, width, ptag, pbufs in (
                    (0, 2048, "psA", 1),
                    (2048, 1024, "psB", 2),
                    (3072, 1024, "psB", 2),
                ):
                    ps = pspool.tile([128, width], f32, tag=ptag, bufs=pbufs)
                    for j in range(width // 512):
                        mlo = mh * MH + off + j * 512
                        nc.tensor.matmul(
                            ps[:, j * 512 : (j + 1) * 512],
                            lhsT=Xa[:, i * 128 : (i + 1) * 128],
                            rhs=Y[:, mlo : mlo + 512],
                            start=True,
                            stop=True,
                        )
                    dve = False
                    if width == 1024:
                        c = cast_counter[0]
                        cast_counter[0] += 1
                        dve = c % 8 in (1, 3) or c < 1
                    if dve and off == 3072:
                        nc.vector.tensor_scalar(
                            out=S[:, off : off + width],
                            in0=ps[:],
                            scalar1=-60000.0,
                            scalar2=None,
                            op0=AL.max,
                            op1=AL.max,
                            accum_out=rowW[:, i, mh * 2 + 1 : mh * 2 + 2],
                        )
                        row_hi = off
                    elif dve:
                        nc.vector.tensor_copy(S[:, off : off + width], ps[:])
                    else:
                        nc.scalar.activation(
                            S[:, off : off + width],
                            ps[:],
                            ACTF.Copy,
                            bias=0.0,
                            scale=1.0,
                        )
                return S, row_hi

            def row_max(S, row_hi, rowW, i, mh):
                scr = scrpool.tile([128, MH], f16, tag="scr")
                nc.vector.tensor_scalar(
                    out=scr[:, 0:row_hi],
                    in0=S[:, 0:row_hi],
                    scalar1=-60000.0,
                    scalar2=None,
                    op0=AL.max,
                    op1=AL.max,
                    accum_out=rowW[:, i, mh * 2 : mh * 2 + 1],
                )

            def scalar_out(red, oidx):
                """red [128, 1] f32 -> outb[0, oidx] via Pool partition sum."""
                pr = rrpool.tile([128, 1], f32, tag="par1")
                nc.gpsimd.partition_all_reduce(pr[:], red[:], channels=128,
                                               reduce_op=RED.add)
                nc.vector.tensor_copy(outb[0:1, oidx : oidx + 1], pr[0:1, 0:1])

            def collapse_sum(acc, oidx):
                """Pool-collapse acc [128, W] f16 over partitions (in 2048-wide
                chunks), then sum over m into outb[0, oidx]."""
                w = acc.shape[1]
                nchunks = (w + 2047) // 2048
                redc = fpool.tile([128, nchunks], f32, tag=f"redc{oidx}")
                for k in range(nchunks):
                    cw = min(2048, w - k * 2048)
                    rr = rrpool.tile([128, 2048], f32, tag="rr")
                    nc.gpsimd.partition_all_reduce(
                        rr[:, 0:cw], acc[:, k * 2048 : k * 2048 + cw],
                        channels=128, reduce_op=RED.max,
                    )
                    nc.vector.tensor_reduce(
                        out=redc[:, k : k + 1], in_=rr[:, 0:cw],
                        axis=mybir.AxisListType.X, op=AL.add,
                    )
                red = fpool.tile([128, 1], f32, tag=f"cred{oidx}")
                nc.vector.tensor_reduce(
                    out=red[:], in_=redc[:], axis=mybir.AxisListType.X, op=AL.add
                )
                nc.vector.tensor_copy(outb[0:1, oidx : oidx + 1], red[0:1, 0:1])

            # ---- unified tile stream: coarse first, then fine; every tile
            # X-routes m-lo fully and the head of m-hi; tails go through
            # tensor_tensor accumulators ----
            first_group = {"f": True, "c": True}

            def flush_group(TG, TG2, colW, fam, nS=GT):
                if first_group[fam]:
                    tgt = colW
                else:
                    tgt = ctpool.tile([128, NJA], f32, tag="colT")
                for j in range(NJ):
                    scr2 = scrpool.tile([128, GT * 128], f16, tag="scrj")
                    nc.vector.tensor_scalar(
                        out=scr2[:, 0 : nS * 128],
                        in0=TG[:, j, 0:nS, :],
                        scalar1=-60000.0,
                        scalar2=None,
                        op0=AL.max,
                        op1=AL.max,
                        accum_out=tgt[:, j : j + 1],
                    )
                for j in range(NJ2):
                    scr2 = scrpool.tile([128, GT * 128], f16, tag="scrj")
                    nc.vector.tensor_scalar(
                        out=scr2[:, 0 : nS * 128],
                        in0=TG2[:, j, 0:nS, :],
                        scalar1=-60000.0,
                        scalar2=None,
                        op0=AL.max,
                        op1=AL.max,
                        accum_out=tgt[:, NJ + j : NJ + j + 1],
                    )
                if tgt is not colW:
                    nc.vector.tensor_tensor(
                        out=colW[:], in0=colW[:], in1=tgt[:], op=AL.max
                    )
                first_group[fam] = False

            tiles = [("c", i) for i in range(n_coarse_tiles)] + [
                ("f", i) for i in range(n_fine_tiles)
            ]
            n_groups_full = len(tiles) // GT
            group_sizes = [GT] * n_groups_full
            assert sum(group_sizes) == len(tiles)
            gidx = [0]
            TG = TG2 = None
            in_group = 0
            step = 0
            pending = []  # (due_step, flush args) delayed for DMA runway
            for fam, i in tiles:
                Xa = Xf if fam == "f" else Xc
                rowW = rowWf if fam == "f" else rowWc
                acc = accF if fam == "f" else accC
                colW = colWF if fam == "f" else colWC
                # m-lo half: X route
                S, rh = make_S(Xa, i, 0, rowW)
                row_max(S, rh, rowW, i, 0)
                if in_group == 0:
                    TG = tgpool.tile([128, NJ, GT, 128], f16, tag="TG")
                    TG2 = tg2pool.tile(
                        [128, NJ2, GT, 128], f16, name="TG2", tag="TG2"
                    )
                nc.sync.dma_start(TG[:, :, in_group, :], S[:], transpose=True)
                # m-hi half: head X-routed, tail tt-routed
                S, rh = make_S(Xa, i, 1, rowW)
                row_max(S, rh, rowW, i, 1)
                nc.sync.dma_start(
                    TG2[:, :, in_group, :], S[:, 0 : NJ2 * 128], transpose=True
                )
                if MT:
                    if i == 0:
                        nc.vector.tensor_copy(acc[:], S[:, NJ2 * 128 : MH])
                    else:
                        nc.vector.tensor_tensor(
                            out=acc[:],
                            in0=acc[:],
                            in1=S[:, NJ2 * 128 : MH],
                            op=AL.max,
                        )
                in_group += 1
                gsz = group_sizes[gidx[0]]
                if in_group == gsz:
                    pending.append(
                        (step + 2 * FLUSH_DELAY, (TG, TG2, colW, fam, gsz))
                    )
                    in_group = 0
                    gidx[0] += 1
                step += 1
                while pending and pending[0][0] <= step:
                    flush_group(*pending.pop(0)[1])
                # mid-stream coarse tail collapse (Pool engine, async)
                if MT and fam == "f" and i == 2:
                    collapse_sum(accC, 7)

            # ---- finals ----
            def row_total(rowW, nT, oidx):
                rmax = fpool.tile([128, nT], f32, tag=f"rmax{oidx}")
                nc.vector.tensor_reduce(
                    out=rmax[:], in_=rowW[:], axis=mybir.AxisListType.X, op=AL.max
                )
                rsum = fpool.tile([128, 1], f32, tag=f"rsum{oidx}")
                nc.vector.tensor_reduce(
                    out=rsum[:], in_=rmax[:], axis=mybir.AxisListType.X, op=AL.add
                )
                scalar_out(rsum, oidx)

            # colW-independent finals first: the trailing group flushes wait
            # on DMA transposes, so this work overlaps them
            if MT:
                collapse_sum(accF, 4)
            else:
                nc.vector.memset(outb[0:1, 4:5], 0.0)
            row_total(rowWf, n_fine_tiles, 0)
            row_total(rowWc, n_coarse_tiles, 2)
            for _, args in pending:
                flush_group(*args)

            # X-route col sums
            cxr = fpool.tile([128, 1], f32, tag="cxr")
            nc.vector.tensor_reduce(
                out=cxr[:], in_=colWF[:], axis=mybir.AxisListType.X, op=AL.add
            )
            scalar_out(cxr, 3)
            cxc = fpool.tile([128, 1], f32, tag="cxc")
            nc.vector.tensor_reduce(
                out=cxc[:], in_=colWC[:], axis=mybir.AxisListType.X, op=AL.add
            )
            scalar_out(cxc, 6)
            if not MT:
                nc.vector.memset(outb[0:1, 7:8], 0.0)

            nc.vector.memset(outb[0:1, 1:2], 0.0)
            nc.vector.memset(outb[0:1, 5:6], 0.0)
            nc.sync.dma_start(out_d.ap(), outb[:])

    nc.compile()
    return nc


def _get_program():
    global _PROGRAM
    if _PROGRAM is None:
        _PROGRAM = _build_program()
    return _PROGRAM


def _aug_x(x_b, n):
    """xaug [7, n]: rows x0,x1,x2, -x2hi, -x2lo, 1, 1 (fp16)."""
    f16 = np.float16
    xa = np.ones((7, n), f16)
    x16 = x_b.astype(f16)
    xa[0:3] = x16.T
    x2 = (x16.astype(np.float32) ** 2).sum(1)
    hi = x2.astype(f16)
    xa[3] = -hi
    xa[4] = -(x2 - hi.astype(np.float32)).astype(f16)
    return xa


def _prep_core_inputs(fine_b, coarse_b, gt_b):
    f16 = np.float16
    g16 = gt_b.astype(f16)  # [3, M]
    yaug = np.ones((7, M), f16)
    yaug[0:3] = (2.0 * g16.astype(np.float32)).astype(f16)
    y2 = (g16.astype(np.float32) ** 2).sum(0)
    hi = y2.astype(f16)
    yaug[5] = -hi
    yaug[6] = -(y2 - hi.astype(np.float32)).astype(f16)
    return {
        "xaug_f": _aug_x(fine_b, NF),
        "xaug_c": _aug_x(coarse_b, NC_),
        "yaug": yaug,
    }


def kernel(coarse, fine, gt, alpha):
    global LAST_RESULTS
    from concourse import bass_utils

    coarse = np.asarray(coarse, np.float32)
    fine = np.asarray(fine, np.float32)
    gt = np.asarray(gt, np.float32)
    alpha = np.float32(np.asarray(alpha))

    nc = _get_program()
    in_maps = [_prep_core_inputs(fine[b], coarse[b], gt[b]) for b in range(B)]
    res = bass_utils.run_bass_kernel_spmd(
        nc, in_maps, core_ids=list(range(B)), trace=PROFILE
    )
    LAST_RESULTS = res
    per = np.stack([r["out"][0] for r in res.results]).astype(np.float64)  # [B, 8]
    # outputs hold NEGATED sums: 0=fine row, 2=coarse row,
    # 3/4=fine col (X part, tail), 6/7=coarse col (X part, tail)
    lf = np.float32((-per[:, 0] / NF - (per[:, 3] + per[:, 4]) / M).mean())
    lc = np.float32((-per[:, 2] / NC_ - (per[:, 6] + per[:, 7]) / M).mean())
    loss = np.float32(lc + np.float32(alpha) * lf)
    return (loss, lc, lf)


if __name__ == "__main__":
    rng = np.random.default_rng(0)
    out = kernel(
        coarse=rng.standard_normal((B, NC_, 3)).astype(np.float32),
        fine=rng.standard_normal((B, NF, 3)).astype(np.float32),
        gt=rng.standard_normal((B, 3, M)).astype(np.float32),
        alpha=np.float32(1.0),
    )
    print(out)


# revision 52
# speedup vs baseline: 1.0338x; 1.0009x over previous
"""Chamfer-distance loss kernel for Trainium2 (8 NeuronCores, SPMD).

Problem: loss = chamfer(coarse, gt_pts) + alpha * chamfer(fine, gt_pts)
  coarse [8,1024,3], fine [8,8192,3], gt [8,3,8192] (channel-first), alpha scalar.
  chamfer(x,y) = mean_n min_m d(n,m) + mean_m min_n d(n,m), d = squared L2.

Sharding: data-parallel over batch - one batch element per NeuronCore.

Per-core pipeline (negated distances, DMA-transposed column path):
  - The PE produces NEGATED distances 128x512 at a time as a K=7 fp16 matmul:
      lhsT rows {x0,x1,x2, -x2hi,-x2lo, 1,1}
      rhs  rows {2y0,2y1,2y2, 1,1, -y2hi,-y2lo}
    so PSUM = -d in fp32 (|x|^2, |y|^2 at ~fp32 precision via fp16 hi/lo
    splits of presummed norms). All mins are maxes; the host negates at the
    end. Negation lets partition collapses use gpsimd partition_all_reduce
    (max) on the otherwise-idle Pool engine (supports add/max/absmax only).
  - Every (tile, m-half) produces S fp16 [128, 4096] via a cast pass split
    between ScalarE (ACTF.Copy) and VectorE (tensor_copy) - the only two
    engines with PSUM access; they are jointly the drain bottleneck.
  - Row direction: one tensor_scalar per (tile, half) at fp16 4x mode
    (op0=max vs -60000 = identity, op1=max into accum_out rowW[:, i, mh]).
  - Col direction:
      * fine m-lo half + head of m-hi: S is DMA-TRANSPOSED (XBAR, idle DMA
        engines) into TG[m_p, j, slot, n] for groups of GT=4 tiles (TG
        double-buffered); per (group, j) one tensor_scalar 4x accum-max over
        free=[4,128] yields uncontaminated per-m maxes at ~0.39 ns/elem
        instead of 0.52 (tensor_tensor 2x). Group flushes are emitted a few
        tiles late to give the DMA runway (engines execute in order).
      * fine m-hi tail + coarse: fp16 2x tensor_tensor running-max into acc
        tiles; partition collapse via Pool partition_all_reduce(max).
  - Scalar totals come from free-axis tensor_reduce(add) + Pool
    partition_all_reduce(add); no PE transposes or ones-matmuls.
  - HW-verified constraints: gpsimd/Pool tensor_tensor/tensor_scalar and DMA
    accum are ISA-rejected on TRN2; matmul psum out must be fp32; DVE 4x
    requires all-SBUF operands (arranged by the transposed-S path);
    partition_all_reduce and XBAR DMA transpose verified correct on HW.

Host does only O(N) prep and the final scalar arithmetic.
"""

import sys

sys.path.insert(0, "/opt/trn_rl_repo")

import numpy as np

B = 8
NF = 8192  # fine points
NC_ = 1024  # coarse points
M = 8192  # gt points
MH = M // 2  # m-half size
NJ = MH // 128  # 32 j-blocks per half

# --- tuning knobs ----------------------------------------------------------
GT = 4  # X-route group size (tiles per transposed group)
TG_BUFS = 2  # transposed-group buffer depth
CAST_DVE_MOD = 4  # every CAST_DVE_MOD-th cast group goes to VectorE
FLUSH_DELAY = 2  # tiles of DMA runway before a group flush
NJ2 = 18  # j-blocks of the m-hi half also X-routed (rest: tt route)

# --- module-level program cache -------------------------------------------
_PROGRAM = None
PROFILE = False
LAST_RESULTS = None


def _build_program():
    from concourse import bacc, bass, tile, bass_isa
    import concourse.mybir as mybir

    f16, f32 = mybir.dt.float16, mybir.dt.float32
    AL = mybir.AluOpType
    ACTF = mybir.ActivationFunctionType
    RED = bass_isa.ReduceOp

    nc = bacc.Bacc("TRN2", target_bir_lowering=False, debug=False, num_devices=B)

    xaug_f = nc.dram_tensor("xaug_f", [7, NF], f16, kind="ExternalInput")
    xaug_c = nc.dram_tensor("xaug_c", [7, NC_], f16, kind="ExternalInput")
    yaug_d = nc.dram_tensor("yaug", [7, M], f16, kind="ExternalInput")
    out_d = nc.dram_tensor("out", [1, 8], f32, kind="ExternalOutput")

    n_fine_tiles = NF // 128  # 64
    n_coarse_tiles = NC_ // 128  # 8
    assert n_fine_tiles % GT == 0

    cast_counter = [0]

    with tile.TileContext(nc) as tc:
        with (
            tc.tile_pool(name="const", bufs=1) as cpool,
            tc.tile_pool(name="s", bufs=4) as spool,
            tc.tile_pool(name="scr", bufs=2) as scrpool,
            tc.tile_pool(name="tg", bufs=TG_BUFS) as tgpool,
            tc.tile_pool(name="tg2", bufs=TG_BUFS) as tg2pool,
            tc.tile_pool(name="ct", bufs=2) as ctpool,
            tc.tile_pool(name="rr", bufs=1) as rrpool,
            tc.tile_pool(name="fin", bufs=1) as fpool,
            tc.tile_pool(name="ps", bufs=2, space=bass.MemorySpace.PSUM) as pspool,
        ):
            warm = cpool.tile([7, 512], f16)
            nc.gpsimd.memset(warm[:], 1.0)
            wsink = cpool.tile([1, 1], f32)
            for _w in range(6):
                wps = pspool.tile([128, 512], f32, name=f"wps{_w}", tag="psB")
                nc.tensor.matmul(wps[:], lhsT=warm[:, 0:128], rhs=warm[:, 0:512],
                                 start=True, stop=True)
                nc.vector.tensor_copy(wsink[:], wps[0:1, 0:1])
            Xf = cpool.tile([7, NF], f16)
            nc.sync.dma_start(Xf[:], xaug_f.ap())
            Xc = cpool.tile([7, NC_], f16)
            nc.sync.dma_start(Xc[:], xaug_c.ap())
            Y = cpool.tile([7, M], f16)
            nc.sync.dma_start(Y[:], yaug_d.ap())

            outb = cpool.tile([1, 8], f32)

            MT = MH - NJ2 * 128  # m-hi tail width kept on the tt route
            NJA = NJ + NJ2  # X-routed j-blocks per tile
            # per-family m-hi-tail col accumulators (tt route)
            accF = cpool.tile([128, MT], f16, name="accF") if MT else None
            accC = cpool.tile([128, MT], f16, name="accC") if MT else None
            rowWf = cpool.tile([128, n_fine_tiles, 4], f32)
            rowWc = cpool.tile([128, n_coarse_tiles, 4], f32)
            nc.vector.memset(rowWf[:], -60000.0)
            nc.vector.memset(rowWc[:], -60000.0)
            colWF = cpool.tile([128, NJA], f32)  # fine X-route per-m col maxes
            colWC = cpool.tile([128, NJA], f32)  # coarse X-route col maxes

            def make_S(Xa, i, mh, rowW):
                """matmuls + cast -> S fp16 [128, MH] (= -d), m-half mh.
                Asymmetric psum slots: one 2048 (single-buffered, always Act)
                + two 1024 (double-buffered ring; DVE-cast eligible) so a
                slow DVE cast never blocks the PE refill rotation. A DVE cast
                on the last 1024 runs as tensor_scalar(max) whose accum_out
                IS that span's row-max for free; returns the remaining row
                span for row_max."""
                S = spool.tile([128, MH], f16, tag="S")
                row_hi = MH
                for off, width, ptag, pbufs in (
                    (0, 2048, "psA", 1),
                    (2048, 1024, "psB", 2),
                    (3072, 1024, "psB", 2),
                ):
                    ps = pspool.tile([128, width], f32, tag=ptag, bufs=pbufs)
                    for j in range(width // 512):
                        mlo = mh * MH + off + j * 512
                        nc.tensor.matmul(
                            ps[:, j * 512 : (j + 1) * 512],
                            lhsT=Xa[:, i * 128 : (i + 1) * 128],
                            rhs=Y[:, mlo : mlo + 512],
                            start=True,
                            stop=True,
                        )
                    dve = False
                    if width == 1024:
                        c = cast_counter[0]
                        cast_counter[0] += 1
                        dve = c % 16 in (1, 3, 5, 9, 11) or c < 1
                    if dve and off == 3072:
                        nc.vector.tensor_scalar(
                            out=S[:, off : off + width],
                            in0=ps[:],
                            scalar1=-60000.0,
                            scalar2=None,
                            op0=AL.max,
                            op1=AL.max,
                            accum_out=rowW[:, i, mh * 2 + 1 : mh * 2 + 2],
                        )
                        row_hi = off
                    elif dve:
                        nc.vector.tensor_copy(S[:, off : off + width], ps[:])
                    else:
                        nc.scalar.activation(
                            S[:, off : off + width],
                            ps[:],
                            ACTF.Copy,
                            bias=0.0,
                            scale=1.0,
                        )
                return S, row_hi

            def row_max(S, row_hi, rowW, i, mh):
                scr = scrpool.tile([128, MH], f16, tag="scr")
                nc.vector.tensor_scalar(
                    out=scr[:, 0:row_hi],
                    in0=S[:, 0:row_hi],
                    scalar1=-60000.0,
                    scalar2=None,
                    op0=AL.max,
                    op1=AL.max,
                    accum_out=rowW[:, i, mh * 2 : mh * 2 + 1],
                )

            def scalar_out(red, oidx):
                """red [128, 1] f32 -> outb[0, oidx] via Pool partition sum."""
                pr = rrpool.tile([128, 1], f32, tag="par1")
                nc.gpsimd.partition_all_reduce(pr[:], red[:], channels=128,
                                               reduce_op=RED.add)
                nc.vector.tensor_copy(outb[0:1, oidx : oidx + 1], pr[0:1, 0:1])

            def collapse_sum(acc, oidx):
                """Pool-collapse acc [128, W] f16 over partitions (in 2048-wide
                chunks), then sum over m into outb[0, oidx]."""
                w = acc.shape[1]
                nchunks = (w + 2047) // 2048
                redc = fpool.tile([128, nchunks], f32, tag=f"redc{oidx}")
                for k in range(nchunks):
                    cw = min(2048, w - k * 2048)
                    rr = rrpool.tile([128, 2048], f32, tag="rr")
                    nc.gpsimd.partition_all_reduce(
                        rr[:, 0:cw], acc[:, k * 2048 : k * 2048 + cw],
                        channels=128, reduce_op=RED.max,
                    )
                    nc.vector.tensor_reduce(
                        out=redc[:, k : k + 1], in_=rr[:, 0:cw],
                        axis=mybir.AxisListType.X, op=AL.add,
                    )
                red = fpool.tile([128, 1], f32, tag=f"cred{oidx}")
                nc.vector.tensor_reduce(
                    out=red[:], in_=redc[:], axis=mybir.AxisListType.X, op=AL.add
                )
                nc.vector.tensor_copy(outb[0:1, oidx : oidx + 1], red[0:1, 0:1])

            # ---- unified tile stream: coarse first, then fine; every tile
            # X-routes m-lo fully and the head of m-hi; tails go through
            # tensor_tensor accumulators ----
            first_group = {"f": True, "c": True}

            def flush_group(TG, TG2, colW, fam, nS=GT):
                if first_group[fam]:
                    tgt = colW
                else:
                    tgt = ctpool.tile([128, NJA], f32, tag="colT")
                for j in range(NJ):
                    scr2 = scrpool.tile([128, GT * 128], f16, tag="scrj")
                    nc.vector.tensor_scalar(
                        out=scr2[:, 0 : nS * 128],
                        in0=TG[:, j, 0:nS, :],
                        scalar1=-60000.0,
                        scalar2=None,
                        op0=AL.max,
                        op1=AL.max,
                        accum_out=tgt[:, j : j + 1],
                    )
                for j in range(NJ2):
                    scr2 = scrpool.tile([128, GT * 128], f16, tag="scrj")
                    nc.vector.tensor_scalar(
                        out=scr2[:, 0 : nS * 128],
                        in0=TG2[:, j, 0:nS, :],
                        scalar1=-60000.0,
                        scalar2=None,
                        op0=AL.max,
                        op1=AL.max,
                        accum_out=tgt[:, NJ + j : NJ + j + 1],
                    )
                if tgt is not colW:
                    nc.vector.tensor_tensor(
                        out=colW[:], in0=colW[:], in1=tgt[:], op=AL.max
                    )
                first_group[fam] = False

            tiles = [("c", i) for i in range(n_coarse_tiles)] + [
                ("f", i) for i in range(n_fine_tiles)
            ]
            n_groups_full = len(tiles) // GT
            group_sizes = [GT] * n_groups_full
            assert sum(group_sizes) == len(tiles)
            gidx = [0]
            TG = TG2 = None
            in_group = 0
            step = 0
            pending = []  # (due_step, flush args) delayed for DMA runway
            for fam, i in tiles:
                Xa = Xf if fam == "f" else Xc
                rowW = rowWf if fam == "f" else rowWc
                acc = accF if fam == "f" else accC
                colW = colWF if fam == "f" else colWC
                # m-lo half: X route
                S, rh = make_S(Xa, i, 0, rowW)
                row_max(S, rh, rowW, i, 0)
                if in_group == 0:
                    TG = tgpool.tile([128, NJ, GT, 128], f16, tag="TG")
                    TG2 = tg2pool.tile(
                        [128, NJ2, GT, 128], f16, name="TG2", tag="TG2"
                    )
                nc.sync.dma_start(TG[:, :, in_group, :], S[:], transpose=True)
                # m-hi half: head X-routed, tail tt-routed
                S, rh = make_S(Xa, i, 1, rowW)
                row_max(S, rh, rowW, i, 1)
                nc.sync.dma_start(
                    TG2[:, :, in_group, :], S[:, 0 : NJ2 * 128], transpose=True
                )
                if MT:
                    if i == 0:
                        nc.vector.tensor_copy(acc[:], S[:, NJ2 * 128 : MH])
                    else:
                        nc.vector.tensor_tensor(
                            out=acc[:],
                            in0=acc[:],
                            in1=S[:, NJ2 * 128 : MH],
                            op=AL.max,
                        )
                in_group += 1
                gsz = group_sizes[gidx[0]]
                if in_group == gsz:
                    pending.append(
                        (step + 2 * FLUSH_DELAY, (TG, TG2, colW, fam, gsz))
                    )
                    in_group = 0
                    gidx[0] += 1
                step += 1
                while pending and pending[0][0] <= step:
                    flush_group(*pending.pop(0)[1])
                # mid-stream coarse tail collapse (Pool engine, async)
                if MT and fam == "f" and i == 2:
                    collapse_sum(accC, 7)

            # ---- finals ----
            def row_total(rowW, nT, oidx):
                rmax = fpool.tile([128, nT], f32, tag=f"rmax{oidx}")
                nc.vector.tensor_reduce(
                    out=rmax[:], in_=rowW[:], axis=mybir.AxisListType.X, op=AL.max
                )
                rsum = fpool.tile([128, 1], f32, tag=f"rsum{oidx}")
                nc.vector.tensor_reduce(
                    out=rsum[:], in_=rmax[:], axis=mybir.AxisListType.X, op=AL.add
                )
                scalar_out(rsum, oidx)

            # colW-independent finals first: the trailing group flushes wait
            # on DMA transposes, so this work overlaps them
            if MT:
                collapse_sum(accF, 4)
            else:
                nc.vector.memset(outb[0:1, 4:5], 0.0)
            row_total(rowWf, n_fine_tiles, 0)
            row_total(rowWc, n_coarse_tiles, 2)
            for _, args in pending:
                flush_group(*args)

            # X-route col sums
            cxr = fpool.tile([128, 1], f32, tag="cxr")
            nc.vector.tensor_reduce(
                out=cxr[:], in_=colWF[:], axis=mybir.AxisListType.X, op=AL.add
            )
            scalar_out(cxr, 3)
            cxc = fpool.tile([128, 1], f32, tag="cxc")
            nc.vector.tensor_reduce(
                out=cxc[:], in_=colWC[:], axis=mybir.AxisListType.X, op=AL.add
            )
            scalar_out(cxc, 6)
            if not MT:
                nc.vector.memset(outb[0:1, 7:8], 0.0)

            nc.vector.memset(outb[0:1, 1:2], 0.0)
            nc.vector.memset(outb[0:1, 5:6], 0.0)
            nc.sync.dma_start(out_d.ap(), outb[:])

    nc.compile()
    return nc


def _get_program():
    global _PROGRAM
    if _PROGRAM is None:
        _PROGRAM = _build_program()
    return _PROGRAM


def _aug_x(x_b, n):
    """xaug [7, n]: rows x0,x1,x2, -x2hi, -x2lo, 1, 1 (fp16)."""
    f16 = np.float16
    xa = np.ones((7, n), f16)
    x16 = x_b.astype(f16)
    xa[0:3] = x16.T
    x2 = (x16.astype(np.float32) ** 2).sum(1)
    hi = x2.astype(f16)
    xa[3] = -hi
    xa[4] = -(x2 - hi.astype(np.float32)).astype(f16)
    return xa


def _prep_core_inputs(fine_b, coarse_b, gt_b):
    f16 = np.float16
    g16 = gt_b.astype(f16)  # [3, M]
    yaug = np.ones((7, M), f16)
    yaug[0:3] = (2.0 * g16.astype(np.float32)).astype(f16)
    y2 = (g16.astype(np.float32) ** 2).sum(0)
    hi = y2.astype(f16)
    yaug[5] = -hi
    yaug[6] = -(y2 - hi.astype(np.float32)).astype(f16)
    return {
        "xaug_f": _aug_x(fine_b, NF),
        "xaug_c": _aug_x(coarse_b, NC_),
        "yaug": yaug,
    }


def kernel(coarse, fine, gt, alpha):
    global LAST_RESULTS
    from concourse import bass_utils

    coarse = np.asarray(coarse, np.float32)
    fine = np.asarray(fine, np.float32)
    gt = np.asarray(gt, np.float32)
    alpha = np.float32(np.asarray(alpha))

    nc = _get_program()
    in_maps = [_prep_core_inputs(fine[b], coarse[b], gt[b]) for b in range(B)]
    res = bass_utils.run_bass_kernel_spmd(
        nc, in_maps, core_ids=list(range(B)), trace=PROFILE
    )
    LAST_RESULTS = res
    per = np.stack([r["out"][0] for r in res.results]).astype(np.float64)  # [B, 8]
    # outputs hold NEGATED sums: 0=fine row, 2=coarse row,
    # 3/4=fine col (X part, tail), 6/7=coarse col (X part, tail)
    lf = np.float32((-per[:, 0] / NF - (per[:, 3] + per[:, 4]) / M).mean())
    lc = np.float32((-per[:, 2] / NC_ - (per[:, 6] + per[:, 7]) / M).mean())
    loss = np.float32(lc + np.float32(alpha) * lf)
    return (loss, lc, lf)


if __name__ == "__main__":
    rng = np.random.default_rng(0)
    out = kernel(
        coarse=rng.standard_normal((B, NC_, 3)).astype(np.float32),
        fine=rng.standard_normal((B, NF, 3)).astype(np.float32),
        gt=rng.standard_normal((B, 3, M)).astype(np.float32),
        alpha=np.float32(1.0),
    )
    print(out)


# revision 53
# speedup vs baseline: 1.0420x; 1.0079x over previous
"""Chamfer-distance loss kernel for Trainium2 (8 NeuronCores, SPMD).

Problem: loss = chamfer(coarse, gt_pts) + alpha * chamfer(fine, gt_pts)
  coarse [8,1024,3], fine [8,8192,3], gt [8,3,8192] (channel-first), alpha scalar.
  chamfer(x,y) = mean_n min_m d(n,m) + mean_m min_n d(n,m), d = squared L2.

Sharding: data-parallel over batch - one batch element per NeuronCore.

Per-core pipeline (negated distances, DMA-transposed column path):
  - The PE produces NEGATED distances 128x512 at a time as a K=7 fp16 matmul:
      lhsT rows {x0,x1,x2, -x2hi,-x2lo, 1,1}
      rhs  rows {2y0,2y1,2y2, 1,1, -y2hi,-y2lo}
    so PSUM = -d in fp32 (|x|^2, |y|^2 at ~fp32 precision via fp16 hi/lo
    splits of presummed norms). All mins are maxes; the host negates at the
    end. Negation lets partition collapses use gpsimd partition_all_reduce
    (max) on the otherwise-idle Pool engine (supports add/max/absmax only).
  - Every (tile, m-half) produces S fp16 [128, 4096] via a cast pass split
    between ScalarE (ACTF.Copy) and VectorE (tensor_copy) - the only two
    engines with PSUM access; they are jointly the drain bottleneck.
  - Row direction: one tensor_scalar per (tile, half) at fp16 4x mode
    (op0=max vs -60000 = identity, op1=max into accum_out rowW[:, i, mh]).
  - Col direction:
      * fine m-lo half + head of m-hi: S is DMA-TRANSPOSED (XBAR, idle DMA
        engines) into TG[m_p, j, slot, n] for groups of GT=4 tiles (TG
        double-buffered); per (group, j) one tensor_scalar 4x accum-max over
        free=[4,128] yields uncontaminated per-m maxes at ~0.39 ns/elem
        instead of 0.52 (tensor_tensor 2x). Group flushes are emitted a few
        tiles late to give the DMA runway (engines execute in order).
      * fine m-hi tail + coarse: fp16 2x tensor_tensor running-max into acc
        tiles; partition collapse via Pool partition_all_reduce(max).
  - Scalar totals come from free-axis tensor_reduce(add) + Pool
    partition_all_reduce(add); no PE transposes or ones-matmuls.
  - HW-verified constraints: gpsimd/Pool tensor_tensor/tensor_scalar and DMA
    accum are ISA-rejected on TRN2; matmul psum out must be fp32; DVE 4x
    requires all-SBUF operands (arranged by the transposed-S path);
    partition_all_reduce and XBAR DMA transpose verified correct on HW.

Host does only O(N) prep and the final scalar arithmetic.
"""

import sys

sys.path.insert(0, "/opt/trn_rl_repo")

import numpy as np

B = 8
NF = 8192  # fine points
NC_ = 1024  # coarse points
M = 8192  # gt points
MH = M // 2  # m-half size
NJ = MH // 128  # 32 j-blocks per half

# --- tuning knobs ----------------------------------------------------------
GT = 4  # X-route group size (tiles per transposed group)
TG_BUFS = 2  # transposed-group buffer depth
CAST_DVE_MOD = 4  # every CAST_DVE_MOD-th cast group goes to VectorE
FLUSH_DELAY = 2  # tiles of DMA runway before a group flush
NJ2 = 18  # j-blocks of the m-hi half also X-routed (rest: tt route)

# --- module-level program cache -------------------------------------------
_PROGRAM = None
PROFILE = False
LAST_RESULTS = None


def _build_program():
    from concourse import bacc, bass, tile, bass_isa
    import concourse.mybir as mybir

    f16, f32 = mybir.dt.float16, mybir.dt.float32
    AL = mybir.AluOpType
    ACTF = mybir.ActivationFunctionType
    RED = bass_isa.ReduceOp

    nc = bacc.Bacc("TRN2", target_bir_lowering=False, debug=False, num_devices=B)

    xaug_f = nc.dram_tensor("xaug_f", [7, NF], f16, kind="ExternalInput")
    xaug_c = nc.dram_tensor("xaug_c", [7, NC_], f16, kind="ExternalInput")
    yaug_d = nc.dram_tensor("yaug", [7, M], f16, kind="ExternalInput")
    out_d = nc.dram_tensor("out", [1, 8], f32, kind="ExternalOutput")

    n_fine_tiles = NF // 128  # 64
    n_coarse_tiles = NC_ // 128  # 8
    assert n_fine_tiles % GT == 0

    cast_counter = [0]

    with tile.TileContext(nc) as tc:
        with (
            tc.tile_pool(name="const", bufs=1) as cpool,
            tc.tile_pool(name="s", bufs=4) as spool,
            tc.tile_pool(name="scr", bufs=2) as scrpool,
            tc.tile_pool(name="tg", bufs=TG_BUFS) as tgpool,
            tc.tile_pool(name="tg2", bufs=TG_BUFS) as tg2pool,
            tc.tile_pool(name="ct", bufs=2) as ctpool,
            tc.tile_pool(name="rr", bufs=1) as rrpool,
            tc.tile_pool(name="fin", bufs=1) as fpool,
            tc.tile_pool(name="ps", bufs=2, space=bass.MemorySpace.PSUM) as pspool,
        ):
            warm = cpool.tile([7, 512], f16)
            nc.gpsimd.memset(warm[:], 1.0)
            wsink = cpool.tile([1, 1], f32)
            for _w in range(6):
                wps = pspool.tile([128, 512], f32, name=f"wps{_w}", tag="psB")
                nc.tensor.matmul(wps[:], lhsT=warm[:, 0:128], rhs=warm[:, 0:512],
                                 start=True, stop=True)
                nc.vector.tensor_copy(wsink[:], wps[0:1, 0:1])
            Xf = cpool.tile([7, NF], f16)
            nc.sync.dma_start(Xf[:], xaug_f.ap())
            Xc = cpool.tile([7, NC_], f16)
            nc.sync.dma_start(Xc[:], xaug_c.ap())
            Y = cpool.tile([7, M], f16)
            nc.sync.dma_start(Y[:], yaug_d.ap())

            outb = cpool.tile([1, 8], f32)

            MT = MH - NJ2 * 128  # m-hi tail width kept on the tt route
            NJA = NJ + NJ2  # X-routed j-blocks per tile
            # per-family m-hi-tail col accumulators (tt route)
            accF = cpool.tile([128, MT], f16, name="accF") if MT else None
            accC = cpool.tile([128, MT], f16, name="accC") if MT else None
            rowWf = cpool.tile([128, n_fine_tiles, 4], f32)
            rowWc = cpool.tile([128, n_coarse_tiles, 4], f32)
            nc.vector.memset(rowWf[:], -60000.0)
            nc.vector.memset(rowWc[:], -60000.0)
            colWF = cpool.tile([128, NJA], f32)  # fine X-route per-m col maxes
            colWC = cpool.tile([128, NJA], f32)  # coarse X-route col maxes

            def make_S(Xa, i, mh, rowW):
                """matmuls + cast -> S fp16 [128, MH] (= -d), m-half mh.
                Asymmetric psum slots: one 2048 (single-buffered, always Act)
                + two 1024 (double-buffered ring; DVE-cast eligible) so a
                slow DVE cast never blocks the PE refill rotation. A DVE cast
                on the last 1024 runs as tensor_scalar(max) whose accum_out
                IS that span's row-max for free; returns the remaining row
                span for row_max."""
                S = spool.tile([128, MH], f16, tag="S")
                row_hi = MH
                for off, width, ptag, pbufs in (
                    (0, 2048, "psA", 1),
                    (2048, 1024, "psB", 2),
                    (3072, 1024, "psB", 2),
                ):
                    ps = pspool.tile([128, width], f32, tag=ptag, bufs=pbufs)
                    for j in range(width // 512):
                        mlo = mh * MH + off + j * 512
                        nc.tensor.matmul(
                            ps[:, j * 512 : (j + 1) * 512],
                            lhsT=Xa[:, i * 128 : (i + 1) * 128],
                            rhs=Y[:, mlo : mlo + 512],
                            start=True,
                            stop=True,
                        )
                    dve = False
                    if width == 1024:
                        c = cast_counter[0]
                        cast_counter[0] += 1
                        dve = c % 16 in (1, 3, 7, 9, 11) or c < 1
                    if dve and off == 3072:
                        nc.vector.tensor_scalar(
                            out=S[:, off : off + width],
                            in0=ps[:],
                            scalar1=-60000.0,
                            scalar2=None,
                            op0=AL.max,
                            op1=AL.max,
                            accum_out=rowW[:, i, mh * 2 + 1 : mh * 2 + 2],
                        )
                        row_hi = off
                    elif dve:
                        nc.vector.tensor_copy(S[:, off : off + width], ps[:])
                    else:
                        nc.scalar.activation(
                            S[:, off : off + width],
                            ps[:],
                            ACTF.Copy,
                            bias=0.0,
                            scale=1.0,
                        )
                return S, row_hi

            def row_max(S, row_hi, rowW, i, mh):
                scr = scrpool.tile([128, MH], f16, tag="scr")
                nc.vector.tensor_scalar(
                    out=scr[:, 0:row_hi],
                    in0=S[:, 0:row_hi],
                    scalar1=-60000.0,
                    scalar2=None,
                    op0=AL.max,
                    op1=AL.max,
                    accum_out=rowW[:, i, mh * 2 : mh * 2 + 1],
                )

            def scalar_out(red, oidx):
                """red [128, 1] f32 -> outb[0, oidx] via Pool partition sum."""
                pr = rrpool.tile([128, 1], f32, tag="par1")
                nc.gpsimd.partition_all_reduce(pr[:], red[:], channels=128,
                                               reduce_op=RED.add)
                nc.vector.tensor_copy(outb[0:1, oidx : oidx + 1], pr[0:1, 0:1])

            def collapse_sum(acc, oidx):
                """Pool-collapse acc [128, W] f16 over partitions (in 2048-wide
                chunks), then sum over m into outb[0, oidx]."""
                w = acc.shape[1]
                nchunks = (w + 2047) // 2048
                redc = fpool.tile([128, nchunks], f32, tag=f"redc{oidx}")
                for k in range(nchunks):
                    cw = min(2048, w - k * 2048)
                    rr = rrpool.tile([128, 2048], f32, tag="rr")
                    nc.gpsimd.partition_all_reduce(
                        rr[:, 0:cw], acc[:, k * 2048 : k * 2048 + cw],
                        channels=128, reduce_op=RED.max,
                    )
                    nc.vector.tensor_reduce(
                        out=redc[:, k : k + 1], in_=rr[:, 0:cw],
                        axis=mybir.AxisListType.X, op=AL.add,
                    )
                red = fpool.tile([128, 1], f32, tag=f"cred{oidx}")
                nc.vector.tensor_reduce(
                    out=red[:], in_=redc[:], axis=mybir.AxisListType.X, op=AL.add
                )
                nc.vector.tensor_copy(outb[0:1, oidx : oidx + 1], red[0:1, 0:1])

            # ---- unified tile stream: coarse first, then fine; every tile
            # X-routes m-lo fully and the head of m-hi; tails go through
            # tensor_tensor accumulators ----
            first_group = {"f": True, "c": True}

            def flush_group(TG, TG2, colW, fam, nS=GT):
                if first_group[fam]:
                    tgt = colW
                else:
                    tgt = ctpool.tile([128, NJA], f32, tag="colT")
                for j in range(NJ):
                    scr2 = scrpool.tile([128, GT * 128], f16, tag="scrj")
                    nc.vector.tensor_scalar(
                        out=scr2[:, 0 : nS * 128],
                        in0=TG[:, j, 0:nS, :],
                        scalar1=-60000.0,
                        scalar2=None,
                        op0=AL.max,
                        op1=AL.max,
                        accum_out=tgt[:, j : j + 1],
                    )
                for j in range(NJ2):
                    scr2 = scrpool.tile([128, GT * 128], f16, tag="scrj")
                    nc.vector.tensor_scalar(
                        out=scr2[:, 0 : nS * 128],
                        in0=TG2[:, j, 0:nS, :],
                        scalar1=-60000.0,
                        scalar2=None,
                        op0=AL.max,
                        op1=AL.max,
                        accum_out=tgt[:, NJ + j : NJ + j + 1],
                    )
                if tgt is not colW:
                    nc.vector.tensor_tensor(
                        out=colW[:], in0=colW[:], in1=tgt[:], op=AL.max
                    )
                first_group[fam] = False

            tiles = [("c", i) for i in range(n_coarse_tiles)] + [
                ("f", i) for i in range(n_fine_tiles)
            ]
            n_groups_full = len(tiles) // GT
            group_sizes = [GT] * n_groups_full
            assert sum(group_sizes) == len(tiles)
            gidx = [0]
            TG = TG2 = None
            in_group = 0
            step = 0
            pending = []  # (due_step, flush args) delayed for DMA runway
            for fam, i in tiles:
                Xa = Xf if fam == "f" else Xc
                rowW = rowWf if fam == "f" else rowWc
                acc = accF if fam == "f" else accC
                colW = colWF if fam == "f" else colWC
                # m-lo half: X route
                S, rh = make_S(Xa, i, 0, rowW)
                row_max(S, rh, rowW, i, 0)
                if in_group == 0:
                    TG = tgpool.tile([128, NJ, GT, 128], f16, tag="TG")
                    TG2 = tg2pool.tile(
                        [128, NJ2, GT, 128], f16, name="TG2", tag="TG2"
                    )
                nc.sync.dma_start(TG[:, :, in_group, :], S[:], transpose=True)
                # m-hi half: head X-routed, tail tt-routed
                S, rh = make_S(Xa, i, 1, rowW)
                row_max(S, rh, rowW, i, 1)
                nc.sync.dma_start(
                    TG2[:, :, in_group, :], S[:, 0 : NJ2 * 128], transpose=True
                )
                if MT:
                    if i == 0:
                        nc.vector.tensor_copy(acc[:], S[:, NJ2 * 128 : MH])
                    else:
                        nc.vector.tensor_tensor(
                            out=acc[:],
                            in0=acc[:],
                            in1=S[:, NJ2 * 128 : MH],
                            op=AL.max,
                        )
                in_group += 1
                gsz = group_sizes[gidx[0]]
                if in_group == gsz:
                    pending.append(
                        (step + 2 * FLUSH_DELAY, (TG, TG2, colW, fam, gsz))
                    )
                    in_group = 0
                    gidx[0] += 1
                step += 1
                while pending and pending[0][0] <= step:
                    flush_group(*pending.pop(0)[1])
                # mid-stream coarse tail collapse (Pool engine, async)
                if MT and fam == "f" and i == 2:
                    collapse_sum(accC, 7)

            # ---- finals ----
            def row_total(rowW, nT, oidx):
                rmax = fpool.tile([128, nT], f32, tag=f"rmax{oidx}")
                nc.vector.tensor_reduce(
                    out=rmax[:], in_=rowW[:], axis=mybir.AxisListType.X, op=AL.max
                )
                rsum = fpool.tile([128, 1], f32, tag=f"rsum{oidx}")
                nc.vector.tensor_reduce(
                    out=rsum[:], in_=rmax[:], axis=mybir.AxisListType.X, op=AL.add
                )
                scalar_out(rsum, oidx)

            # colW-independent finals first: the trailing group flushes wait
            # on DMA transposes, so this work overlaps them
            if MT:
                collapse_sum(accF, 4)
            else:
                nc.vector.memset(outb[0:1, 4:5], 0.0)
            row_total(rowWf, n_fine_tiles, 0)
            row_total(rowWc, n_coarse_tiles, 2)
            for _, args in pending:
                flush_group(*args)

            # X-route col sums
            cxr = fpool.tile([128, 1], f32, tag="cxr")
            nc.vector.tensor_reduce(
                out=cxr[:], in_=colWF[:], axis=mybir.AxisListType.X, op=AL.add
            )
            scalar_out(cxr, 3)
            cxc = fpool.tile([128, 1], f32, tag="cxc")
            nc.vector.tensor_reduce(
                out=cxc[:], in_=colWC[:], axis=mybir.AxisListType.X, op=AL.add
            )
            scalar_out(cxc, 6)
            if not MT:
                nc.vector.memset(outb[0:1, 7:8], 0.0)

            nc.vector.memset(outb[0:1, 1:2], 0.0)
            nc.vector.memset(outb[0:1, 5:6], 0.0)
            nc.sync.dma_start(out_d.ap(), outb[:])

    nc.compile()
    return nc


def _get_program():
    global _PROGRAM
    if _PROGRAM is None:
        _PROGRAM = _build_program()
    return _PROGRAM


def _aug_x(x_b, n):
    """xaug [7, n]: rows x0,x1,x2, -x2hi, -x2lo, 1, 1 (fp16)."""
    f16 = np.float16
    xa = np.ones((7, n), f16)
    x16 = x_b.astype(f16)
    xa[0:3] = x16.T
    x2 = (x16.astype(np.float32) ** 2).sum(1)
    hi = x2.astype(f16)
    xa[3] = -hi
    xa[4] = -(x2 - hi.astype(np.float32)).astype(f16)
    return xa


def _prep_core_inputs(fine_b, coarse_b, gt_b):
    f16 = np.float16
    g16 = gt_b.astype(f16)  # [3, M]
    yaug = np.ones((7, M), f16)
    yaug[0:3] = (2.0 * g16.astype(np.float32)).astype(f16)
    y2 = (g16.astype(np.float32) ** 2).sum(0)
    hi = y2.astype(f16)
    yaug[5] = -hi
    yaug[6] = -(y2 - hi.astype(np.float32)).astype(f16)
    return {
        "xaug_f": _aug_x(fine_b, NF),
        "xaug_c": _aug_x(coarse_b, NC_),
        "yaug": yaug,
    }


def kernel(coarse, fine, gt, alpha):
    global LAST_RESULTS
    from concourse import bass_utils

    coarse = np.asarray(coarse, np.float32)
    fine = np.asarray(fine, np.float32)
    gt = np.asarray(gt, np.float32)
    alpha = np.float32(np.asarray(alpha))

    nc = _get_program()
    in_maps = [_prep_core_inputs(fine[b], coarse[b], gt[b]) for b in range(B)]
    res = bass_utils.run_bass_kernel_spmd(
        nc, in_maps, core_ids=list(range(B)), trace=PROFILE
    )
    LAST_RESULTS = res
    per = np.stack([r["out"][0] for r in res.results]).astype(np.float64)  # [B, 8]
    # outputs hold NEGATED sums: 0=fine row, 2=coarse row,
    # 3/4=fine col (X part, tail), 6/7=coarse col (X part, tail)
    lf = np.float32((-per[:, 0] / NF - (per[:, 3] + per[:, 4]) / M).mean())
    lc = np.float32((-per[:, 2] / NC_ - (per[:, 6] + per[:, 7]) / M).mean())
    loss = np.float32(lc + np.float32(alpha) * lf)
    return (loss, lc, lf)


if __name__ == "__main__":
    rng = np.random.default_rng(0)
    out = kernel(
        coarse=rng.standard_normal((B, NC_, 3)).astype(np.float32),
        fine=rng.standard_normal((B, NF, 3)).astype(np.float32),
        gt=rng.standard_normal((B, 3, M)).astype(np.float32),
        alpha=np.float32(1.0),
    )
    print(out)


# revision 54
# speedup vs baseline: 1.0489x; 1.0066x over previous
"""Chamfer-distance loss kernel for Trainium2 (8 NeuronCores, SPMD).

Problem: loss = chamfer(coarse, gt_pts) + alpha * chamfer(fine, gt_pts)
  coarse [8,1024,3], fine [8,8192,3], gt [8,3,8192] (channel-first), alpha scalar.
  chamfer(x,y) = mean_n min_m d(n,m) + mean_m min_n d(n,m), d = squared L2.

Sharding: data-parallel over batch - one batch element per NeuronCore.

Per-core pipeline (negated distances, DMA-transposed column path):
  - The PE produces NEGATED distances 128x512 at a time as a K=7 fp16 matmul:
      lhsT rows {x0,x1,x2, -x2hi,-x2lo, 1,1}
      rhs  rows {2y0,2y1,2y2, 1,1, -y2hi,-y2lo}
    so PSUM = -d in fp32 (|x|^2, |y|^2 at ~fp32 precision via fp16 hi/lo
    splits of presummed norms). All mins are maxes; the host negates at the
    end. Negation lets partition collapses use gpsimd partition_all_reduce
    (max) on the otherwise-idle Pool engine (supports add/max/absmax only).
  - Every (tile, m-half) produces S fp16 [128, 4096] via a cast pass split
    between ScalarE (ACTF.Copy) and VectorE (tensor_copy) - the only two
    engines with PSUM access; they are jointly the drain bottleneck.
  - Row direction: one tensor_scalar per (tile, half) at fp16 4x mode
    (op0=max vs -60000 = identity, op1=max into accum_out rowW[:, i, mh]).
  - Col direction:
      * fine m-lo half + head of m-hi: S is DMA-TRANSPOSED (XBAR, idle DMA
        engines) into TG[m_p, j, slot, n] for groups of GT=4 tiles (TG
        double-buffered); per (group, j) one tensor_scalar 4x accum-max over
        free=[4,128] yields uncontaminated per-m maxes at ~0.39 ns/elem
        instead of 0.52 (tensor_tensor 2x). Group flushes are emitted a few
        tiles late to give the DMA runway (engines execute in order).
      * fine m-hi tail + coarse: fp16 2x tensor_tensor running-max into acc
        tiles; partition collapse via Pool partition_all_reduce(max).
  - Scalar totals come from free-axis tensor_reduce(add) + Pool
    partition_all_reduce(add); no PE transposes or ones-matmuls.
  - HW-verified constraints: gpsimd/Pool tensor_tensor/tensor_scalar and DMA
    accum are ISA-rejected on TRN2; matmul psum out must be fp32; DVE 4x
    requires all-SBUF operands (arranged by the transposed-S path);
    partition_all_reduce and XBAR DMA transpose verified correct on HW.

Host does only O(N) prep and the final scalar arithmetic.
"""

import sys

sys.path.insert(0, "/opt/trn_rl_repo")

import numpy as np

B = 8
NF = 8192  # fine points
NC_ = 1024  # coarse points
M = 8192  # gt points
MH = M // 2  # m-half size
NJ = MH // 128  # 32 j-blocks per half

# --- tuning knobs ----------------------------------------------------------
GT = 4  # X-route group size (tiles per transposed group)
TG_BUFS = 2  # transposed-group buffer depth
CAST_DVE_MOD = 4  # every CAST_DVE_MOD-th cast group goes to VectorE
FLUSH_DELAY = 2  # tiles of DMA runway before a group flush
NJ2 = 18  # j-blocks of the m-hi half also X-routed (rest: tt route)

# --- module-level program cache -------------------------------------------
_PROGRAM = None
PROFILE = False
LAST_RESULTS = None


def _build_program():
    from concourse import bacc, bass, tile, bass_isa
    import concourse.mybir as mybir

    f16, f32 = mybir.dt.float16, mybir.dt.float32
    AL = mybir.AluOpType
    ACTF = mybir.ActivationFunctionType
    RED = bass_isa.ReduceOp

    nc = bacc.Bacc("TRN2", target_bir_lowering=False, debug=False, num_devices=B)

    xaug_f = nc.dram_tensor("xaug_f", [7, NF], f16, kind="ExternalInput")
    xaug_c = nc.dram_tensor("xaug_c", [7, NC_], f16, kind="ExternalInput")
    yaug_d = nc.dram_tensor("yaug", [7, M], f16, kind="ExternalInput")
    out_d = nc.dram_tensor("out", [1, 8], f32, kind="ExternalOutput")

    n_fine_tiles = NF // 128  # 64
    n_coarse_tiles = NC_ // 128  # 8
    assert n_fine_tiles % GT == 0

    cast_counter = [0]

    with tile.TileContext(nc) as tc:
        with (
            tc.tile_pool(name="const", bufs=1) as cpool,
            tc.tile_pool(name="s", bufs=4) as spool,
            tc.tile_pool(name="scr", bufs=2) as scrpool,
            tc.tile_pool(name="tg", bufs=TG_BUFS) as tgpool,
            tc.tile_pool(name="tg2", bufs=TG_BUFS) as tg2pool,
            tc.tile_pool(name="ct", bufs=2) as ctpool,
            tc.tile_pool(name="rr", bufs=1) as rrpool,
            tc.tile_pool(name="fin", bufs=1) as fpool,
            tc.tile_pool(name="ps", bufs=2, space=bass.MemorySpace.PSUM) as pspool,
        ):
            warm = cpool.tile([7, 512], f16)
            nc.gpsimd.memset(warm[:], 1.0)
            wsink = cpool.tile([1, 1], f32)
            for _w in range(6):
                wps = pspool.tile([128, 512], f32, name=f"wps{_w}", tag="psB")
                nc.tensor.matmul(wps[:], lhsT=warm[:, 0:128], rhs=warm[:, 0:512],
                                 start=True, stop=True)
                nc.vector.tensor_copy(wsink[:], wps[0:1, 0:1])
            Xf = cpool.tile([7, NF], f16)
            nc.sync.dma_start(Xf[:], xaug_f.ap())
            Xc = cpool.tile([7, NC_], f16)
            nc.sync.dma_start(Xc[:], xaug_c.ap())
            Y = cpool.tile([7, M], f16)
            nc.sync.dma_start(Y[:], yaug_d.ap())

            outb = cpool.tile([1, 8], f32)

            MT = MH - NJ2 * 128  # m-hi tail width kept on the tt route
            NJA = NJ + NJ2  # X-routed j-blocks per tile
            # per-family m-hi-tail col accumulators (tt route)
            accF = cpool.tile([128, MT], f16, name="accF") if MT else None
            accC = cpool.tile([128, MT], f16, name="accC") if MT else None
            rowWf = cpool.tile([128, n_fine_tiles, 4], f32)
            rowWc = cpool.tile([128, n_coarse_tiles, 4], f32)
            nc.vector.memset(rowWf[:], -60000.0)
            nc.vector.memset(rowWc[:], -60000.0)
            colWF = cpool.tile([128, NJA], f32)  # fine X-route per-m col maxes
            colWC = cpool.tile([128, NJA], f32)  # coarse X-route col maxes

            def make_S(Xa, i, mh, rowW):
                """matmuls + cast -> S fp16 [128, MH] (= -d), m-half mh.
                Asymmetric psum slots: one 2048 (single-buffered, always Act)
                + two 1024 (double-buffered ring; DVE-cast eligible) so a
                slow DVE cast never blocks the PE refill rotation. A DVE cast
                on the last 1024 runs as tensor_scalar(max) whose accum_out
                IS that span's row-max for free; returns the remaining row
                span for row_max."""
                S = spool.tile([128, MH], f16, tag="S")
                row_hi = MH
                for off, width, ptag, pbufs in (
                    (0, 2048, "psA", 1),
                    (2048, 1024, "psB", 2),
                    (3072, 1024, "psB", 2),
                ):
                    ps = pspool.tile([128, width], f32, tag=ptag, bufs=pbufs)
                    for j in range(width // 512):
                        mlo = mh * MH + off + j * 512
                        nc.tensor.matmul(
                            ps[:, j * 512 : (j + 1) * 512],
                            lhsT=Xa[:, i * 128 : (i + 1) * 128],
                            rhs=Y[:, mlo : mlo + 512],
                            start=True,
                            stop=True,
                        )
                    dve = False
                    if width == 1024:
                        c = cast_counter[0]
                        cast_counter[0] += 1
                        dve = (c % 2 == 1 if c < 16 else c % 16 in (1, 3, 7, 9, 11)) or c < 1
                    if dve and off == 3072:
                        nc.vector.tensor_scalar(
                            out=S[:, off : off + width],
                            in0=ps[:],
                            scalar1=-60000.0,
                            scalar2=None,
                            op0=AL.max,
                            op1=AL.max,
                            accum_out=rowW[:, i, mh * 2 + 1 : mh * 2 + 2],
                        )
                        row_hi = off
                    elif dve:
                        nc.vector.tensor_copy(S[:, off : off + width], ps[:])
                    else:
                        nc.scalar.activation(
                            S[:, off : off + width],
                            ps[:],
                            ACTF.Copy,
                            bias=0.0,
                            scale=1.0,
                        )
                return S, row_hi

            def row_max(S, row_hi, rowW, i, mh):
                scr = scrpool.tile([128, MH], f16, tag="scr")
                nc.vector.tensor_scalar(
                    out=scr[:, 0:row_hi],
                    in0=S[:, 0:row_hi],
                    scalar1=-60000.0,
                    scalar2=None,
                    op0=AL.max,
                    op1=AL.max,
                    accum_out=rowW[:, i, mh * 2 : mh * 2 + 1],
                )

            def scalar_out(red, oidx):
                """red [128, 1] f32 -> outb[0, oidx] via Pool partition sum."""
                pr = rrpool.tile([128, 1], f32, tag="par1")
                nc.gpsimd.partition_all_reduce(pr[:], red[:], channels=128,
                                               reduce_op=RED.add)
                nc.vector.tensor_copy(outb[0:1, oidx : oidx + 1], pr[0:1, 0:1])

            def collapse_sum(acc, oidx):
                """Pool-collapse acc [128, W] f16 over partitions (in 2048-wide
                chunks), then sum over m into outb[0, oidx]."""
                w = acc.shape[1]
                nchunks = (w + 2047) // 2048
                redc = fpool.tile([128, nchunks], f32, tag=f"redc{oidx}")
                for k in range(nchunks):
                    cw = min(2048, w - k * 2048)
                    rr = rrpool.tile([128, 2048], f32, tag="rr")
                    nc.gpsimd.partition_all_reduce(
                        rr[:, 0:cw], acc[:, k * 2048 : k * 2048 + cw],
                        channels=128, reduce_op=RED.max,
                    )
                    nc.vector.tensor_reduce(
                        out=redc[:, k : k + 1], in_=rr[:, 0:cw],
                        axis=mybir.AxisListType.X, op=AL.add,
                    )
                red = fpool.tile([128, 1], f32, tag=f"cred{oidx}")
                nc.vector.tensor_reduce(
                    out=red[:], in_=redc[:], axis=mybir.AxisListType.X, op=AL.add
                )
                nc.vector.tensor_copy(outb[0:1, oidx : oidx + 1], red[0:1, 0:1])

            # ---- unified tile stream: coarse first, then fine; every tile
            # X-routes m-lo fully and the head of m-hi; tails go through
            # tensor_tensor accumulators ----
            first_group = {"f": True, "c": True}

            def flush_group(TG, TG2, colW, fam, nS=GT):
                if first_group[fam]:
                    tgt = colW
                else:
                    tgt = ctpool.tile([128, NJA], f32, tag="colT")
                for j in range(NJ):
                    scr2 = scrpool.tile([128, GT * 128], f16, tag="scrj")
                    nc.vector.tensor_scalar(
                        out=scr2[:, 0 : nS * 128],
                        in0=TG[:, j, 0:nS, :],
                        scalar1=-60000.0,
                        scalar2=None,
                        op0=AL.max,
                        op1=AL.max,
                        accum_out=tgt[:, j : j + 1],
                    )
                for j in range(NJ2):
                    scr2 = scrpool.tile([128, GT * 128], f16, tag="scrj")
                    nc.vector.tensor_scalar(
                        out=scr2[:, 0 : nS * 128],
                        in0=TG2[:, j, 0:nS, :],
                        scalar1=-60000.0,
                        scalar2=None,
                        op0=AL.max,
                        op1=AL.max,
                        accum_out=tgt[:, NJ + j : NJ + j + 1],
                    )
                if tgt is not colW:
                    nc.vector.tensor_tensor(
                        out=colW[:], in0=colW[:], in1=tgt[:], op=AL.max
                    )
                first_group[fam] = False

            tiles = [("c", i) for i in range(n_coarse_tiles)] + [
                ("f", i) for i in range(n_fine_tiles)
            ]
            n_groups_full = len(tiles) // GT
            group_sizes = [GT] * n_groups_full
            assert sum(group_sizes) == len(tiles)
            gidx = [0]
            TG = TG2 = None
            in_group = 0
            step = 0
            pending = []  # (due_step, flush args) delayed for DMA runway
            for fam, i in tiles:
                Xa = Xf if fam == "f" else Xc
                rowW = rowWf if fam == "f" else rowWc
                acc = accF if fam == "f" else accC
                colW = colWF if fam == "f" else colWC
                # m-lo half: X route
                S, rh = make_S(Xa, i, 0, rowW)
                row_max(S, rh, rowW, i, 0)
                if in_group == 0:
                    TG = tgpool.tile([128, NJ, GT, 128], f16, tag="TG")
                    TG2 = tg2pool.tile(
                        [128, NJ2, GT, 128], f16, name="TG2", tag="TG2"
                    )
                nc.sync.dma_start(TG[:, :, in_group, :], S[:], transpose=True)
                # m-hi half: head X-routed, tail tt-routed
                S, rh = make_S(Xa, i, 1, rowW)
                row_max(S, rh, rowW, i, 1)
                nc.sync.dma_start(
                    TG2[:, :, in_group, :], S[:, 0 : NJ2 * 128], transpose=True
                )
                if MT:
                    if i == 0:
                        nc.vector.tensor_copy(acc[:], S[:, NJ2 * 128 : MH])
                    else:
                        nc.vector.tensor_tensor(
                            out=acc[:],
                            in0=acc[:],
                            in1=S[:, NJ2 * 128 : MH],
                            op=AL.max,
                        )
                in_group += 1
                gsz = group_sizes[gidx[0]]
                if in_group == gsz:
                    pending.append(
                        (step + 2 * FLUSH_DELAY, (TG, TG2, colW, fam, gsz))
                    )
                    in_group = 0
                    gidx[0] += 1
                step += 1
                while pending and pending[0][0] <= step:
                    flush_group(*pending.pop(0)[1])
                # mid-stream coarse tail collapse (Pool engine, async)
                if MT and fam == "f" and i == 2:
                    collapse_sum(accC, 7)

            # ---- finals ----
            def row_total(rowW, nT, oidx):
                rmax = fpool.tile([128, nT], f32, tag=f"rmax{oidx}")
                nc.vector.tensor_reduce(
                    out=rmax[:], in_=rowW[:], axis=mybir.AxisListType.X, op=AL.max
                )
                rsum = fpool.tile([128, 1], f32, tag=f"rsum{oidx}")
                nc.vector.tensor_reduce(
                    out=rsum[:], in_=rmax[:], axis=mybir.AxisListType.X, op=AL.add
                )
                scalar_out(rsum, oidx)

            # colW-independent finals first: the trailing group flushes wait
            # on DMA transposes, so this work overlaps them
            if MT:
                collapse_sum(accF, 4)
            else:
                nc.vector.memset(outb[0:1, 4:5], 0.0)
            row_total(rowWf, n_fine_tiles, 0)
            row_total(rowWc, n_coarse_tiles, 2)
            for _, args in pending:
                flush_group(*args)

            # X-route col sums
            cxr = fpool.tile([128, 1], f32, tag="cxr")
            nc.vector.tensor_reduce(
                out=cxr[:], in_=colWF[:], axis=mybir.AxisListType.X, op=AL.add
            )
            scalar_out(cxr, 3)
            cxc = fpool.tile([128, 1], f32, tag="cxc")
            nc.vector.tensor_reduce(
                out=cxc[:], in_=colWC[:], axis=mybir.AxisListType.X, op=AL.add
            )
            scalar_out(cxc, 6)
            if not MT:
                nc.vector.memset(outb[0:1, 7:8], 0.0)

            nc.vector.memset(outb[0:1, 1:2], 0.0)
            nc.vector.memset(outb[0:1, 5:6], 0.0)
            nc.sync.dma_start(out_d.ap(), outb[:])

    nc.compile()
    return nc


def _get_program():
    global _PROGRAM
    if _PROGRAM is None:
        _PROGRAM = _build_program()
    return _PROGRAM


def _aug_x(x_b, n):
    """xaug [7, n]: rows x0,x1,x2, -x2hi, -x2lo, 1, 1 (fp16)."""
    f16 = np.float16
    xa = np.ones((7, n), f16)
    x16 = x_b.astype(f16)
    xa[0:3] = x16.T
    x2 = (x16.astype(np.float32) ** 2).sum(1)
    hi = x2.astype(f16)
    xa[3] = -hi
    xa[4] = -(x2 - hi.astype(np.float32)).astype(f16)
    return xa


def _prep_core_inputs(fine_b, coarse_b, gt_b):
    f16 = np.float16
    g16 = gt_b.astype(f16)  # [3, M]
    yaug = np.ones((7, M), f16)
    yaug[0:3] = (2.0 * g16.astype(np.float32)).astype(f16)
    y2 = (g16.astype(np.float32) ** 2).sum(0)
    hi = y2.astype(f16)
    yaug[5] = -hi
    yaug[6] = -(y2 - hi.astype(np.float32)).astype(f16)
    return {
        "xaug_f": _aug_x(fine_b, NF),
        "xaug_c": _aug_x(coarse_b, NC_),
        "yaug": yaug,
    }


def kernel(coarse, fine, gt, alpha):
    global LAST_RESULTS
    from concourse import bass_utils

    coarse = np.asarray(coarse, np.float32)
    fine = np.asarray(fine, np.float32)
    gt = np.asarray(gt, np.float32)
    alpha = np.float32(np.asarray(alpha))

    nc = _get_program()
    in_maps = [_prep_core_inputs(fine[b], coarse[b], gt[b]) for b in range(B)]
    res = bass_utils.run_bass_kernel_spmd(
        nc, in_maps, core_ids=list(range(B)), trace=PROFILE
    )
    LAST_RESULTS = res
    per = np.stack([r["out"][0] for r in res.results]).astype(np.float64)  # [B, 8]
    # outputs hold NEGATED sums: 0=fine row, 2=coarse row,
    # 3/4=fine col (X part, tail), 6/7=coarse col (X part, tail)
    lf = np.float32((-per[:, 0] / NF - (per[:, 3] + per[:, 4]) / M).mean())
    lc = np.float32((-per[:, 2] / NC_ - (per[:, 6] + per[:, 7]) / M).mean())
    loss = np.float32(lc + np.float32(alpha) * lf)
    return (loss, lc, lf)


if __name__ == "__main__":
    rng = np.random.default_rng(0)
    out = kernel(
        coarse=rng.standard_normal((B, NC_, 3)).astype(np.float32),
        fine=rng.standard_normal((B, NF, 3)).astype(np.float32),
        gt=rng.standard_normal((B, 3, M)).astype(np.float32),
        alpha=np.float32(1.0),
    )
    print(out)
